# revision 1
# baseline (speedup 1.0000x reference)
"""Trainium2 Bass kernel for CanonCausalMultiheadAttn.

Sharding: tensor-parallel over heads across 8 cores (2 q-heads + 1 kv-head
per core), both batches replicated. Each core computes its heads' attention
for both batches, then a single 8-core AllToAll exchanges attention outputs
so each core owns one (batch, seq-slice) of the final output projection.

Per-core pipeline (all shapes hardcoded for B=2, S=2048, D=2048):
  QKV proj (bf16 matmul) -> canon conv chunk-wise (DVE) -> qk rmsnorm via PE
  column-sum matmuls; q's rstd broadcast via K=1 outer-product matmul, k's
  rstd applied later as the per-partition `scale` of the exp activation ->
  RoPE (DVE, norm-weight & 1/sqrt(dh) folded into host cos/sin tables) ->
  causal attention with scores in [Sk, Sq] layout (bf16 matmul; no
  max-subtraction needed since |logit| <= sqrt(128) after qk-norm) ->
  exp (ACT, bf16 out) -> P@V with a ones-column appended to V giving row
  sums for free -> AllToAll -> output projection (bf16 matmul).
"""
import sys

sys.path.insert(0, '/opt/trn_rl_repo')

import numpy as np
import ml_dtypes

import concourse.bass as bass
import concourse.mybir as mybir
import concourse.tile as tile
from concourse import bacc
from concourse.bass_utils import run_bass_kernel_spmd

F32 = mybir.dt.float32
F32R = mybir.dt.float32r
BF16 = mybir.dt.bfloat16
AF = mybir.ActivationFunctionType
ALU = mybir.AluOpType

B, S, D = 2, 2048, 2048
NH, NKV, DH = 16, 8, 128
K_CONV = 4
EPS = 1e-6
SCALE = 1.0 / float(np.sqrt(DH))
NEG = -1e9
N_CORES = 8
N_CHUNKS = S // 512     # 512-wide chunks per batch
DEBUG = False
N_SKB = S // 128        # Sk blocks per batch
VSTR = 144              # V_aug stride per Sk block; 144*2B = 288B keeps each
                        # block 32B-aligned for the xbar DMA transpose


def _build():
    nc = bacc.Bacc("TRN2", target_bir_lowering=False, debug=False,
                   num_devices=N_CORES)

    hsT = nc.dram_tensor("hsT", [D, B * S], BF16, kind="ExternalInput")
    wT = nc.dram_tensor("wT", [D, 512], BF16, kind="ExternalInput")
    woT = nc.dram_tensor("woT", [D, D], BF16, kind="ExternalInput")
    cw = nc.dram_tensor("cw", [512, K_CONV], F32, kind="ExternalInput")
    ropeAq = nc.dram_tensor("ropeAq", [DH, S], F32, kind="ExternalInput")
    ropeBq = nc.dram_tensor("ropeBq", [DH, S], F32, kind="ExternalInput")
    ropeAk = nc.dram_tensor("ropeAk", [DH, S], F32, kind="ExternalInput")
    ropeBk = nc.dram_tensor("ropeBk", [DH, S], F32, kind="ExternalInput")
    maskd = nc.dram_tensor("maskd", [128, 128], F32, kind="ExternalInput")
    out = nc.dram_tensor("out", [512, D], F32, kind="ExternalOutput")
    dbg = {}
    if DEBUG:
        dbg["cn0"] = nc.dram_tensor("d_cn0", [128, S], F32, kind="ExternalOutput")
        dbg["cn3"] = nc.dram_tensor("d_cn3", [128, S], BF16, kind="ExternalOutput")
        dbg["roped0"] = nc.dram_tensor("d_roped0", [128, S], BF16, kind="ExternalOutput")
        dbg["roped2"] = nc.dram_tensor("d_roped2", [128, S], BF16, kind="ExternalOutput")
        dbg["rstdkT"] = nc.dram_tensor("d_rstdkT", [128, N_SKB], F32, kind="ExternalOutput")
        dbg["vaug"] = nc.dram_tensor("d_vaug", [128, N_SKB * VSTR], BF16, kind="ExternalOutput")
        dbg["bc"] = nc.dram_tensor("d_bc", [128, S], F32, kind="ExternalOutput")
        dbg["p00"] = nc.dram_tensor("d_p00", [128, 512], BF16, kind="ExternalOutput")
        dbg["ab0"] = nc.dram_tensor("d_ab0", [128, 128], BF16, kind="ExternalOutput")
        dbg["a2ain"] = nc.dram_tensor("d_a2ain", [2048, 512], BF16, kind="ExternalOutput")
        dbg["a2aout"] = nc.dram_tensor("d_a2aout", [2048, 512], BF16, kind="ExternalOutput")

    with tile.TileContext(nc) as tc:
        with tc.tile_pool(name="const", bufs=1) as cpool, \
             tc.tile_pool(name="persist", bufs=1) as pers, \
             tc.tile_pool(name="dram", bufs=1, space="DRAM") as dram:

            # ---- constants ----
            ropes = {}
            for nm, t in (("Aq", ropeAq), ("Bq", ropeBq),
                          ("Ak", ropeAk), ("Bk", ropeBk)):
                rt = cpool.tile([DH, S], F32, tag=f"rope{nm}", name=f"rope{nm}")
                nc.sync.dma_start(rt[:], t.ap())
                ropes[nm] = rt
            mask_sb = cpool.tile([128, 128], F32, tag="mask")
            nc.sync.dma_start(mask_sb[:], maskd.ap())
            cw_sb = []
            for mt in range(4):
                t = cpool.tile([128, K_CONV], F32, tag=f"cw{mt}", name=f"cw{mt}")
                nc.sync.dma_start(t[:], cw.ap()[128 * mt:128 * mt + 128, :])
                cw_sb.append(t)
            ones_col_f = cpool.tile([128, 1], F32, tag="ocf")
            nc.vector.memset(ones_col_f[:], 1.0)
            ones_col = cpool.tile([128, 1], F32R, tag="oc")
            nc.scalar.copy(ones_col[:], ones_col_f[:])
            eps_sb = cpool.tile([1, 1], F32, tag="eps")
            nc.vector.memset(eps_sb[:], EPS)
            ones_row_f = cpool.tile([1, 128], F32, tag="orf")
            nc.vector.memset(ones_row_f[:], 1.0)
            ones_row = cpool.tile([1, 128], F32R, tag="or")
            nc.scalar.copy(ones_row[:], ones_row_f[:])
            s0_sb = []
            for mt in range(4):
                t = cpool.tile([128, 1], F32, tag=f"s0{mt}", name=f"s0{mt}")
                nc.vector.tensor_scalar_add(t[:], cw_sb[mt][:, 0:1], 1.0)
                s0_sb.append(t)

            # persistent per-(b,mt) tiles
            roped = {}   # (b, mt<3) -> [128, S] bf16
            vaug = {}    # b -> [128, N_SKB*VSTR] bf16
            rstdkT = {}  # b -> [128, N_SKB] f32 (k rstd, transposed per block)

            # ============ QKV + canon + norm + rope, per batch ============
            for b in range(B):
                with tc.tile_pool(name=f"bwork{b}", bufs=1) as bw:
                    cn = {}
                    for mt in range(3):
                        cn[mt] = bw.tile([128, S], F32, tag=f"cn{mt}",
                                         name=f"cn{mt}")
                    cn[3] = bw.tile([128, S], BF16, tag="cn3", name="cn3")
                    with tc.tile_pool(name=f"qps{b}", bufs=2,
                                      space="PSUM") as qps:
                        prev_raw = {}
                        for n in range(N_CHUNKS):
                            hs_sb = bw.tile([128, 16 * 512], BF16,
                                            tag="hschunk", bufs=2,
                                            name="hs_sb")
                            nc.sync.dma_start(
                                hs_sb[:].rearrange("p (k s) -> p k s", s=512),
                                hsT.ap()[:, b * S + 512 * n:
                                         b * S + 512 * (n + 1)]
                                .rearrange("(k p) s -> p k s", p=128))
                            hv = hs_sb[:].rearrange("p (k s) -> p k s", s=512)
                            psums = [qps.tile([128, 512], F32, tag=f"qk{mt}",
                                              name=f"qk{mt}")
                                     for mt in range(4)]
                            for k in range(16):
                                wt_k = bw.tile([128, 512], BF16, tag="wtk",
                                               bufs=6, name="wt_k")
                                nc.sync.dma_start(
                                    wt_k[:],
                                    wT.ap()[128 * k:128 * (k + 1), :])
                                for mt in range(4):
                                    nc.tensor.matmul(
                                        psums[mt][:],
                                        wt_k[:, 128 * mt:128 * (mt + 1)],
                                        hv[:, k, :],
                                        start=(k == 0), stop=(k == 15))
                            # canon conv, chunk-wise from a raw copy
                            for mt in range(4):
                                raw_c = bw.tile([128, 512], F32,
                                                tag=f"rawc{mt}", bufs=2,
                                                name=f"rawc{mt}")
                                nc.scalar.copy(raw_c[:], psums[mt][:])
                                c = cn[mt]
                                lo = 512 * n
                                nc.vector.tensor_scalar_mul(
                                    c[:, lo:lo + 512], raw_c[:], s0_sb[mt][:])
                                for k in range(1, K_CONV):
                                    nc.vector.scalar_tensor_tensor(
                                        c[:, lo + k:lo + 512],
                                        raw_c[:, 0:512 - k],
                                        cw_sb[mt][:, k:k + 1],
                                        c[:, lo + k:lo + 512],
                                        ALU.mult, ALU.add)
                                    if n > 0:
                                        nc.vector.scalar_tensor_tensor(
                                            c[:, lo:lo + k],
                                            prev_raw[mt][:, 512 - k:512],
                                            cw_sb[mt][:, k:k + 1],
                                            c[:, lo:lo + k],
                                            ALU.mult, ALU.add)
                                prev_raw[mt] = raw_c

                    # V: transpose canon output into V_aug blocks
                    va = pers.tile([128, N_SKB * VSTR], BF16, tag=f"vaug{b}",
                                   name=f"vaug{b}")
                    vaug[b] = va
                    for i in range(N_SKB):
                        nc.sync.dma_start_transpose(
                            va[:, VSTR * i:VSTR * i + 128],
                            cn[3][:, 128 * i:128 * (i + 1)])
                    nc.vector.memset(
                        va[:].rearrange("p (i c) -> p i c",
                                        c=VSTR)[:, :, 128:129], 1.0)

                    # rmsnorm rstd + rope for q0, q1, k
                    rkt = pers.tile([128, N_SKB], F32, tag=f"rstdkT{b}",
                                    name=f"rstdkT{b}")
                    rstdkT[b] = rkt
                    rk_d = dram.tile([N_SKB, 128], F32, tag=f"rkd{b}",
                                     name=f"rk_d{b}")
                    with tc.tile_pool(name=f"nps{b}", bufs=2,
                                      space="PSUM") as nps, \
                         tc.tile_pool(name=f"bps{b}", bufs=2,
                                      space="PSUM") as bps:
                        for mt in range(3):
                            x = cn[mt]
                            is_q = mt < 2
                            bc = None
                            if is_q:
                                bc = bw.tile([128, S], F32, tag="bc",
                                             name="bc")
                            for c in range(N_CHUNKS):
                                sq = bw.tile([128, 512], F32R, tag="sqr",
                                             bufs=2, name="sq")
                                nc.vector.tensor_mul(
                                    sq[:], x[:, 512 * c:512 * (c + 1)],
                                    x[:, 512 * c:512 * (c + 1)])
                                sp = nps.tile([1, 512], F32, tag="ssq")
                                nc.tensor.matmul(sp[:], ones_col[:], sq[:],
                                                 start=True, stop=True)
                                srt = bw.tile([1, 512], F32, tag="srt",
                                              bufs=2, name="srt")
                                nc.scalar.activation(srt[:], sp[:], AF.Sqrt,
                                                     bias=eps_sb[:],
                                                     scale=1.0 / DH)
                                if is_q:
                                    rq = bw.tile([1, 512], F32R, tag="rq",
                                                 bufs=2, name="rq")
                                    with nc.allow_low_precision(
                                            reason="rstd f32r ample"):
                                        nc.vector.reciprocal(rq[:], srt[:])
                                    bp = bps.tile([128, 512], F32, tag="bcp")
                                    nc.tensor.matmul(bp[:], ones_row[:],
                                                     rq[:], start=True,
                                                     stop=True)
                                    nc.scalar.copy(
                                        bc[:, 512 * c:512 * (c + 1)], bp[:])
                                else:
                                    rk = bw.tile([1, 512], F32, tag="rk",
                                                 bufs=2, name="rk")
                                    nc.vector.reciprocal(rk[:], srt[:])
                                    nc.sync.dma_start(
                                        rk_d[4 * c:4 * (c + 1), :], rk[:])
                            if mt == 2:
                                nc.sync.dma_start(
                                    rkt[:],
                                    rk_d[:].rearrange("i p -> p i"))
                            # rope: roped = (x*A + shift64(x)*B) [* bc for q]
                            A_ = ropes["Aq"] if is_q else ropes["Ak"]
                            B_ = ropes["Bq"] if is_q else ropes["Bk"]
                            sh = bw.tile([128, S], F32, tag="shift",
                                         name="sh")
                            nc.sync.dma_start(sh[0:64, :], x[64:128, :])
                            nc.sync.dma_start(sh[64:128, :], x[0:64, :])
                            nc.vector.tensor_mul(sh[:], sh[:], B_[:])
                            tm = bw.tile([128, S], F32, tag="ropetmp",
                                         name="tm")
                            nc.vector.tensor_mul(tm[:], x[:], A_[:])
                            ro = pers.tile([128, S], BF16,
                                           tag=f"roped{b}{mt}",
                                           name=f"roped{b}{mt}")
                            if is_q:
                                nc.vector.tensor_add(tm[:], tm[:], sh[:])
                                nc.vector.tensor_mul(ro[:], tm[:], bc[:])
                            else:
                                nc.vector.tensor_add(ro[:], tm[:], sh[:])
                            roped[(b, mt)] = ro
                            if DEBUG and b == 0 and is_q and mt == 0:
                                nc.sync.dma_start(dbg["bc"].ap(), bc[:])
                    if DEBUG and b == 0:
                        nc.sync.dma_start(dbg["cn0"].ap(), cn[0][:])
                        nc.sync.dma_start(dbg["cn3"].ap(), cn[3][:])
                        nc.sync.dma_start(dbg["roped0"].ap(), roped[(0, 0)][:])
                        nc.sync.dma_start(dbg["roped2"].ap(), roped[(0, 2)][:])
                        nc.sync.dma_start(dbg["rstdkT"].ap(), rstdkT[0][:])
                        nc.sync.dma_start(dbg["vaug"].ap(), vaug[0][:])

            # ======================= attention =======================
            a2a_in = dram.tile([2048, 512], BF16, tag="a2ain", name="a2ain")
            a2a_out = dram.tile([2048, 512], BF16, tag="a2aout",
                                name="a2aout")

            with tc.tile_pool(name="scps", bufs=3, space="PSUM") as scps, \
                 tc.tile_pool(name="atps", bufs=4, space="PSUM") as atps, \
                 tc.tile_pool(name="apool", bufs=1) as apool:
                for b in range(B):
                    KT = roped[(b, 2)]
                    va = vaug[b]
                    rkt = rstdkT[b]
                    for h in range(2):
                        QT = roped[(b, h)]
                        for j in range(N_CHUNKS):
                            ptiles = []
                            for i in range(4 * j + 4):
                                r = i - 4 * j
                                off = 128 * max(r, 0)
                                ps = scps.tile([128, 512], F32, tag="sc",
                                               name="ps")
                                nc.tensor.matmul(
                                    ps[:, off:512],
                                    KT[:, 128 * i:128 * (i + 1)],
                                    QT[:, 512 * j + off:512 * (j + 1)],
                                    start=True, stop=True)
                                if r >= 0:
                                    nc.vector.tensor_add(
                                        ps[:, off:off + 128],
                                        ps[:, off:off + 128], mask_sb[:])
                                pt = apool.tile([128, 512], BF16, tag="p",
                                                bufs=18, name="pt")
                                nc.scalar.activation(
                                    pt[:, off:512], ps[:, off:512], AF.Exp,
                                    scale=rkt[:, i:i + 1])
                                if (DEBUG and b == 0 and h == 0
                                        and j == 0 and i == 0):
                                    nc.sync.dma_start(dbg["p00"].ap(), pt[:])
                                ptiles.append(pt)
                            for mp in range(4):
                                mg = 4 * j + mp
                                at = atps.tile([128, VSTR], F32, tag="at",
                                               name="at")
                                for i in range(mg + 1):
                                    nc.tensor.matmul(
                                        at[:, 0:129],
                                        ptiles[i][:, 128 * mp:128 * (mp + 1)],
                                        va[:, VSTR * i:VSTR * i + 129],
                                        start=(i == 0), stop=(i == mg))
                                rec = apool.tile([128, 1], F32, tag="rec",
                                                 bufs=3, name="rec")
                                nc.vector.reciprocal(rec[:], at[:, 128:129])
                                ab = apool.tile([128, 128], BF16, tag="ab",
                                                bufs=3, name="ab")
                                nc.vector.tensor_scalar_mul(
                                    ab[:], at[:, 0:128], rec[:])
                                if (DEBUG and b == 0 and h == 0
                                        and mg == 0):
                                    nc.sync.dma_start(dbg["ab0"].ap(), ab[:])
                                att = apool.tile([128, 128], BF16, tag="att",
                                                 bufs=3, name="att")
                                nc.sync.dma_start_transpose(att[:], ab[:])
                                rd = 4 * b + (mg // 4)
                                nc.sync.dma_start(
                                    a2a_in[256 * rd + 128 * h:
                                           256 * rd + 128 * (h + 1),
                                           128 * (mg % 4):128 * (mg % 4 + 1)],
                                    att[:])

            # ======================= all-to-all =======================
            nc.gpsimd.collective_compute(
                "AllToAll", ALU.bypass,
                replica_groups=[list(range(N_CORES))],
                ins=[a2a_in.opt()], outs=[a2a_out.opt()],
                cc_dim="Partition")

            if DEBUG:
                nc.sync.dma_start(dbg["a2ain"].ap(), a2a_in[:])
                nc.sync.dma_start(dbg["a2aout"].ap(), a2a_out[:])

            # ====================== out projection ====================
            with tc.tile_pool(name="opool", bufs=1) as opool, \
                 tc.tile_pool(name="ops", bufs=2, space="PSUM") as ops:
                aout = opool.tile([128, 16 * 512], BF16, tag="aout")
                nc.sync.dma_start(
                    aout[:].rearrange("p (k s) -> p k s", s=512),
                    a2a_out[:].rearrange("(k p) s -> p k s", p=128))
                av = aout[:].rearrange("p (k s) -> p k s", s=512)
                for n in range(4):
                    pso = [ops.tile([128, 512], F32, tag=f"o{m}",
                                    name=f"o{m}") for m in range(4)]
                    for k in range(16):
                        wo_t = opool.tile([128, 512], BF16, tag="wo", bufs=6,
                                          name="wo_t")
                        nc.sync.dma_start(
                            wo_t[:],
                            woT.ap()[128 * k:128 * (k + 1),
                                     512 * n:512 * (n + 1)])
                        for mp in range(4):
                            nc.tensor.matmul(
                                pso[mp][:],
                                av[:, k, 128 * mp:128 * (mp + 1)],
                                wo_t[:], start=(k == 0), stop=(k == 15))
                    for mp in range(4):
                        os_t = opool.tile([128, 512], F32, tag="osb", bufs=3,
                                          name="os_t")
                        nc.scalar.copy(os_t[:], pso[mp][:])
                        nc.sync.dma_start(
                            out.ap()[128 * mp:128 * (mp + 1),
                                     512 * n:512 * (n + 1)], os_t[:])

    nc.compile()
    return nc


_NC_CACHE = None


def _get_nc():
    global _NC_CACHE
    if _NC_CACHE is None:
        _NC_CACHE = _build()
    return _NC_CACHE


def _host_prep(inputs):
    hs = np.asarray(inputs["hidden_states"], dtype=np.float32)
    Wq = np.asarray(inputs["Wq"], dtype=np.float32)
    Wk = np.asarray(inputs["Wk"], dtype=np.float32)
    Wv = np.asarray(inputs["Wv"], dtype=np.float32)
    Wo = np.asarray(inputs["Wo"], dtype=np.float32)
    cqw = np.asarray(inputs["canon_q_w"], dtype=np.float32)
    ckw = np.asarray(inputs["canon_k_w"], dtype=np.float32)
    cvw = np.asarray(inputs["canon_v_w"], dtype=np.float32)
    qnw = np.asarray(inputs["q_norm_w"], dtype=np.float32)
    knw = np.asarray(inputs["k_norm_w"], dtype=np.float32)

    bf = ml_dtypes.bfloat16
    hsT = np.ascontiguousarray(
        np.concatenate([hs[0].T, hs[1].T], axis=1)).astype(bf)
    WqT, WkT, WvT = Wq.T, Wk.T, Wv.T
    woT = np.ascontiguousarray(Wo.T).astype(bf)

    inv_freq = 1.0 / (10000.0 ** (np.arange(0, DH, 2, dtype=np.float64) / DH))
    freqs = np.arange(S, dtype=np.float64)[:, None] * inv_freq
    emb = np.concatenate([freqs, freqs], axis=-1)
    cosT, sinT = np.cos(emb).T, np.sin(emb).T

    def make_rope(normw, scale):
        A = cosT * normw[:, None] * scale
        wswap = normw[(np.arange(DH) + 64) % DH]
        sign = np.where(np.arange(DH) < 64, -1.0, 1.0)
        Bc = sinT * wswap[:, None] * sign[:, None] * scale
        return (np.ascontiguousarray(A).astype(np.float32),
                np.ascontiguousarray(Bc).astype(np.float32))

    Aq, Bq = make_rope(qnw, SCALE)
    Ak, Bk = make_rope(knw, 1.0)

    p = np.arange(128)[:, None]
    f = np.arange(128)[None, :]
    maskd = np.where(p <= f, 0.0, NEG).astype(np.float32)

    in_maps = []
    for r in range(N_CORES):
        wTc = np.ascontiguousarray(np.concatenate(
            [WqT[:, 256 * r:256 * r + 256],
             WkT[:, 128 * r:128 * r + 128],
             WvT[:, 128 * r:128 * r + 128]], axis=1)).astype(bf)
        cwc = np.ascontiguousarray(np.concatenate(
            [cqw[256 * r:256 * r + 256],
             ckw[128 * r:128 * r + 128],
             cvw[128 * r:128 * r + 128]], axis=0)).astype(np.float32)
        in_maps.append({
            "hsT": hsT, "wT": wTc, "woT": woT, "cw": cwc,
            "ropeAq": Aq, "ropeBq": Bq, "ropeAk": Ak, "ropeBk": Bk,
            "maskd": maskd,
        })
    return in_maps


def kernel(**inputs):
    nc = _get_nc()
    in_maps = _host_prep(inputs)
    res = run_bass_kernel_spmd(nc, in_maps, core_ids=list(range(N_CORES)))
    full = np.empty((B, S, D), np.float32)
    for r in range(N_CORES):
        full[r // 4, 512 * (r % 4):512 * (r % 4 + 1), :] = res.results[r]["out"]
    return full



# revision 6
# speedup vs baseline: 1.2104x; 1.2104x over previous
"""Trainium2 Bass kernel for CanonCausalMultiheadAttn.

Sharding: tensor-parallel over heads across 8 cores (2 q-heads + 1 kv-head
per core), both batches replicated. Two head-split AllToAlls exchange
attention outputs so each core owns one (batch, seq-slice) of the final
output projection; the first overlaps the second half of attention and the
second overlaps the h=0 part of the output projection.

Per-core pipeline (shapes hardcoded for B=2, S=2048, D=2048):
  QKV proj (bf16 matmul, weights SBUF-resident) -> canon conv via halo'd
  raw buffer (DVE, bf16 2x) -> qk rmsnorm via PE column-sum matmuls ->
  RoPE (DVE bf16; norm-weight & 1/sqrt(dh) folded into host tables; q rstd
  broadcast via K=1 matmul, k rstd folded into the exp scale) -> causal
  attention with scores in [Sk, Sq] layout; PV computed transposed
  (V stationary) giving output directly in [dh, Sq]; softmax denominator
  via ones-column matmuls -> normalize -> AllToAll x2 (head-split,
  Shared outputs) -> output projection (bf16 matmul, Wo SBUF-resident).
"""
import sys

sys.path.insert(0, '/opt/trn_rl_repo')

import numpy as np
import ml_dtypes

import concourse.bass as bass
import concourse.mybir as mybir
import concourse.tile as tile
from concourse import bacc
from concourse.bass_utils import run_bass_kernel_spmd

F32 = mybir.dt.float32
F32R = mybir.dt.float32r
BF16 = mybir.dt.bfloat16
AF = mybir.ActivationFunctionType
ALU = mybir.AluOpType

B, S, D = 2, 2048, 2048
NH, NKV, DH = 16, 8, 128
K_CONV = 4
EPS = 1e-6
SCALE = 1.0 / float(np.sqrt(DH))
NEG = -1e9
N_CORES = 8
NCB = S // 512          # 512-token chunks per batch
N_SKB = S // 128        # Sk blocks per batch


def _build():
    nc = bacc.Bacc("TRN2", target_bir_lowering=False, debug=False,
                   num_devices=N_CORES)

    hsT = nc.dram_tensor("hsT", [D, B * S], BF16, kind="ExternalInput")
    wT = nc.dram_tensor("wT", [D, 512], BF16, kind="ExternalInput")
    woT = nc.dram_tensor("woT", [D, D], BF16, kind="ExternalInput")
    cw = nc.dram_tensor("cw", [512, K_CONV], F32, kind="ExternalInput")
    ropeAq = nc.dram_tensor("ropeAq", [DH, S], BF16, kind="ExternalInput")
    ropeBq = nc.dram_tensor("ropeBq", [DH, S], BF16, kind="ExternalInput")
    ropeAk = nc.dram_tensor("ropeAk", [DH, S], BF16, kind="ExternalInput")
    ropeBk = nc.dram_tensor("ropeBk", [DH, S], BF16, kind="ExternalInput")
    maskd = nc.dram_tensor("maskd", [128, 128], F32, kind="ExternalInput")
    out = nc.dram_tensor("out", [512, D], F32, kind="ExternalOutput")

    with tile.TileContext(nc) as tc:
        with tc.tile_pool(name="const", bufs=1) as cpool, \
             tc.tile_pool(name="persist", bufs=1) as pers, \
             tc.tile_pool(name="dram", bufs=1, space="DRAM") as dram:

            # ---- constants ----
            ropes = {}
            for nm, t in (("Aq", ropeAq), ("Bq", ropeBq),
                          ("Ak", ropeAk), ("Bk", ropeBk)):
                rt = cpool.tile([DH, S], BF16, tag=f"rope{nm}",
                                name=f"rope{nm}")
                nc.sync.dma_start(rt[:], t.ap())
                ropes[nm] = rt
            mask_sb = cpool.tile([128, 128], F32, tag="mask")
            nc.sync.dma_start(mask_sb[:], maskd.ap())
            cw_sb = []
            for mt in range(4):
                t = cpool.tile([128, K_CONV], F32, tag=f"cw{mt}",
                               name=f"cw{mt}")
                nc.sync.dma_start(t[:], cw.ap()[128 * mt:128 * mt + 128, :])
                cw_sb.append(t)
            ones_col = cpool.tile([128, 1], BF16, tag="oc")
            nc.vector.memset(ones_col[:], 1.0)
            eps_sb = cpool.tile([1, 1], F32, tag="eps")
            nc.vector.memset(eps_sb[:], EPS)
            ones_row_f = cpool.tile([1, 128], F32, tag="orf")
            nc.vector.memset(ones_row_f[:], 1.0)
            ones_row = cpool.tile([1, 128], F32R, tag="or")
            nc.scalar.copy(ones_row[:], ones_row_f[:])
            s0_sb = []
            for mt in range(4):
                t = cpool.tile([128, 1], F32, tag=f"s0{mt}", name=f"s0{mt}")
                nc.vector.tensor_scalar_add(t[:], cw_sb[mt][:, 0:1], 1.0)
                s0_sb.append(t)

            # QKV weights resident in SBUF: [128, 16 k-blocks x 512]
            wT_sb = cpool.tile([128, 16 * 512], BF16, tag="wTsb")
            nc.sync.dma_start(
                wT_sb[:].rearrange("p (k c) -> p k c", c=512),
                wT.ap().rearrange("(k p) c -> p k c", p=128))

            # persistent per-(b,mt) tiles
            roped = {}   # (b, mt<3) -> [128, S] bf16
            vT = {}      # b -> [128, N_SKB*128] bf16 (V transposed blocks)
            rstdkT = {}  # b -> [128, N_SKB] f32
            rq_sb = {}   # (b, mt<2) -> [1, S] f32r

            for b in range(B):
                vT[b] = pers.tile([128, N_SKB * 128], BF16, tag=f"vT{b}",
                                  name=f"vT{b}")
                rstdkT[b] = pers.tile([128, N_SKB], F32, tag=f"rstdkT{b}",
                                      name=f"rstdkT{b}")
                for mt in range(3):
                    roped[(b, mt)] = pers.tile(
                        [128, S], BF16, tag=f"roped{b}{mt}",
                        name=f"roped{b}{mt}")
                for mt in range(2):
                    rq_sb[(b, mt)] = pers.tile(
                        [1, S], F32R, tag=f"rq{b}{mt}", name=f"rq{b}{mt}")

            # ============ phase Q: QKV + canon + norm + rope ============
            with tc.tile_pool(name="qps", bufs=1, space="PSUM") as qps, \
                 tc.tile_pool(name="spp", bufs=2, space="PSUM") as spp, \
                 tc.tile_pool(name="bps", bufs=2, space="PSUM") as bps:
                for b in range(B):
                    rk_d = dram.tile([N_SKB, 128], F32, tag=f"rkd{b}",
                                     name=f"rk_d{b}")
                    with tc.tile_pool(name=f"bwork{b}", bufs=1) as bw:
                        cn = {}
                        raw_h = {}
                        for mt in range(4):
                            cn[mt] = bw.tile([128, S], BF16, tag=f"cn{mt}",
                                             name=f"cn{mt}")
                            raw_h[mt] = bw.tile([128, 516], BF16,
                                                tag=f"rawh{mt}",
                                                name=f"raw_h{mt}")
                            nc.vector.memset(raw_h[mt][:, 0:4], 0.0)
                        psums = [qps.tile([128, 512], F32, tag=f"qk{mt}",
                                          name=f"qk{mt}") for mt in range(4)]

                        def emit_chunk_mms(n):
                            hs_sb = bw.tile([128, 16 * 512], BF16,
                                            tag="hschunk", bufs=2,
                                            name="hs_sb")
                            nc.sync.dma_start(
                                hs_sb[:].rearrange("p (k s) -> p k s", s=512),
                                hsT.ap()[:, b * S + 512 * n:
                                         b * S + 512 * (n + 1)]
                                .rearrange("(k p) s -> p k s", p=128))
                            hv = hs_sb[:].rearrange("p (k s) -> p k s", s=512)
                            wv = wT_sb[:].rearrange("p (k c) -> p k c", c=512)
                            for k in range(16):
                                for mt in range(4):
                                    nc.tensor.matmul(
                                        psums[mt][:],
                                        wv[:, k, 128 * mt:128 * (mt + 1)],
                                        hv[:, k, :],
                                        start=(k == 0), stop=(k == 15))

                        def emit_canon(n):
                            lo = 512 * n
                            for mt in range(4):
                                rh = raw_h[mt]
                                if n > 0:
                                    nc.vector.tensor_copy(
                                        rh[:, 1:4], rh[:, 513:516])
                                nc.scalar.copy(rh[:, 4:516], psums[mt][:])
                                c = cn[mt]
                                nc.vector.tensor_scalar_mul(
                                    c[:, lo:lo + 512], rh[:, 4:516],
                                    s0_sb[mt][:])
                                for k in range(1, K_CONV):
                                    nc.vector.scalar_tensor_tensor(
                                        c[:, lo:lo + 512],
                                        rh[:, 4 - k:516 - k],
                                        cw_sb[mt][:, k:k + 1],
                                        c[:, lo:lo + 512],
                                        ALU.mult, ALU.add)
                            # squares for rmsnorm (q0, q1, k)
                            for mt in range(3):
                                sq = bw.tile([128, 512], BF16, tag="sqr",
                                             bufs=3, name="sq")
                                nc.vector.tensor_mul(
                                    sq[:], cn[mt][:, lo:lo + 512],
                                    cn[mt][:, lo:lo + 512])
                                sqs[(n, mt)] = sq

                        def emit_norm(n):
                            for mt in range(3):
                                sp = spp.tile([1, 512], F32, tag="ssq")
                                nc.tensor.matmul(sp[:], ones_col[:],
                                                 sqs.pop((n, mt))[:],
                                                 start=True, stop=True)
                                srt = bw.tile([1, 512], F32, tag="srt",
                                              bufs=2, name="srt")
                                nc.scalar.activation(srt[:], sp[:], AF.Sqrt,
                                                     bias=eps_sb[:],
                                                     scale=1.0 / DH)
                                if mt < 2:
                                    with nc.allow_low_precision(
                                            reason="rstd f32r ample"):
                                        nc.vector.reciprocal(
                                            rq_sb[(b, mt)]
                                            [:, 512 * n:512 * (n + 1)],
                                            srt[:])
                                else:
                                    rk = bw.tile([1, 512], F32, tag="rk",
                                                 bufs=2, name="rk")
                                    nc.vector.reciprocal(rk[:], srt[:])
                                    nc.sync.dma_start(
                                        rk_d[4 * n:4 * (n + 1), :], rk[:])
                            # V transpose blocks for this chunk
                            for t in range(4):
                                i = 4 * n + t
                                nc.sync.dma_start_transpose(
                                    vT[b][:, 128 * i:128 * (i + 1)],
                                    cn[3][:, 128 * i:128 * (i + 1)])

                        sqs = {}
                        for n in range(NCB + 1):
                            if n < NCB:
                                emit_chunk_mms(n)
                            if n >= 1:
                                emit_norm(n - 1)
                            if n < NCB:
                                emit_canon(n)
                        nc.sync.dma_start(
                            rstdkT[b][:], rk_d[:].rearrange("i p -> p i"))

                        # rope
                        for mt in range(3):
                            is_q = mt < 2
                            x = cn[mt]
                            A_ = ropes["Aq"] if is_q else ropes["Ak"]
                            B_ = ropes["Bq"] if is_q else ropes["Bk"]
                            sh = bw.tile([128, S], BF16, tag="shift",
                                         bufs=2, name="sh")
                            nc.sync.dma_start(sh[0:64, :], x[64:128, :])
                            nc.sync.dma_start(sh[64:128, :], x[0:64, :])
                            nc.vector.tensor_mul(sh[:], sh[:], B_[:])
                            tm = bw.tile([128, S], BF16, tag="ropetmp",
                                         bufs=2, name="tm")
                            nc.vector.tensor_mul(tm[:], x[:], A_[:])
                            ro = roped[(b, mt)]
                            if is_q:
                                nc.vector.tensor_add(tm[:], tm[:], sh[:])
                                for c in range(NCB):
                                    bp = bps.tile([128, 512], F32, tag="bcp")
                                    nc.tensor.matmul(
                                        bp[:], ones_row[:],
                                        rq_sb[(b, mt)]
                                        [:, 512 * c:512 * (c + 1)],
                                        start=True, stop=True)
                                    bcb = bw.tile([128, 512], BF16,
                                                  tag="bcb", bufs=2,
                                                  name="bcb")
                                    nc.scalar.copy(bcb[:], bp[:])
                                    nc.vector.tensor_mul(
                                        ro[:, 512 * c:512 * (c + 1)],
                                        tm[:, 512 * c:512 * (c + 1)],
                                        bcb[:])
                            else:
                                nc.vector.tensor_add(ro[:], tm[:], sh[:])

            # ============ attention + head-split all-to-all ============
            wpool_ctx = tc.tile_pool(name="wpool", bufs=1)
            wpool = wpool_ctx.__enter__()
            # Wo resident prefetch (needed only for the output projection)
            wo_sb = wpool.tile([128, 16 * D], BF16, tag="wosb")
            wov = wo_sb[:].rearrange("p (g o) -> p g o", o=D)
            for gg in range(4):
                nc.sync.dma_start(
                    wov[:, 4 * gg:4 * (gg + 1), :],
                    woT.ap()[512 * gg:512 * (gg + 1), :]
                    .rearrange("(g p) o -> p g o", p=128))

            a2a_in = {}
            a2a_out = {}
            oin = {}
            for h in range(2):
                a2a_in[h] = dram.tile([1024, 512], BF16, tag=f"a2ain{h}",
                                      name=f"a2a_in{h}")
                a2a_out[h] = dram.tile([1024, 512], BF16, tag=f"a2aout{h}",
                                       name=f"a2a_out{h}")
                oin[h] = wpool.tile([128, 8 * 512], BF16, tag=f"oin{h}",
                                    name=f"oin{h}")

            with tc.tile_pool(name="scps", bufs=3, space="PSUM") as scps, \
                 tc.tile_pool(name="pvps", bufs=2, space="PSUM") as pvps, \
                 tc.tile_pool(name="dnps", bufs=2, space="PSUM") as dnps, \
                 tc.tile_pool(name="bcps", bufs=1, space="PSUM") as bcps, \
                 tc.tile_pool(name="apool", bufs=1) as apool:
                for h in range(2):
                    for b in range(B):
                        KT = roped[(b, 2)]
                        QT = roped[(b, h)]
                        vt = vT[b]
                        rkt = rstdkT[b]
                        for j in range(NCB):
                            pv = pvps.tile([128, 512], F32, tag="pv",
                                           name="pv")
                            dn = dnps.tile([1, 512], F32, tag="dn",
                                           name="dn")
                            nb = 4 * j + 4
                            pts = [None] * nb

                            def emit_qk(i):
                                r = i - 4 * j
                                off = 128 * max(r, 0)
                                ps = scps.tile([128, 512], F32, tag="sc",
                                               name="ps")
                                nc.tensor.matmul(
                                    ps[:, off:512],
                                    KT[:, 128 * i:128 * (i + 1)],
                                    QT[:, 512 * j + off:512 * (j + 1)],
                                    start=True, stop=True)
                                if r >= 0:
                                    nc.vector.tensor_add(
                                        ps[:, off:off + 128],
                                        ps[:, off:off + 128], mask_sb[:])
                                pt = apool.tile([128, 512], BF16, tag="p",
                                                bufs=4, name="pt")
                                if r > 0:
                                    nc.vector.memset(pt[:, 0:off], 0.0)
                                nc.scalar.activation(
                                    pt[:, off:512], ps[:, off:512], AF.Exp,
                                    scale=rkt[:, i:i + 1])
                                pts[i] = pt

                            def emit_pv(i):
                                first = (i == 0)
                                last = (i == nb - 1)
                                nc.tensor.matmul(
                                    pv[:], vt[:, 128 * i:128 * (i + 1)],
                                    pts[i][:], start=first, stop=last)
                                nc.tensor.matmul(
                                    dn[:], ones_col[:], pts[i][:],
                                    start=first, stop=last)

                            for i in range(nb):
                                emit_qk(i)
                                if i >= 1:
                                    emit_pv(i - 1)
                            emit_pv(nb - 1)

                            rec = apool.tile([1, 512], F32R, tag="rec",
                                             bufs=2, name="rec")
                            with nc.allow_low_precision(
                                    reason="softmax denom f32r ample"):
                                nc.vector.reciprocal(rec[:], dn[:])
                            bc = bcps.tile([128, 512], F32, tag="bc",
                                           name="bc")
                            nc.tensor.matmul(bc[:], ones_row[:], rec[:],
                                             start=True, stop=True)
                            bcb = apool.tile([128, 512], BF16, tag="bcbn",
                                             bufs=2, name="bcb")
                            nc.vector.tensor_copy(bcb[:], bc[:])
                            nrm = apool.tile([128, 512], BF16, tag="nrm",
                                             bufs=2, name="nrm")
                            nc.vector.tensor_mul(nrm[:], pv[:], bcb[:])
                            nc.sync.dma_start(
                                a2a_in[h][128 * (4 * b + j):
                                          128 * (4 * b + j + 1), :],
                                nrm[:])
                    nc.gpsimd.collective_compute(
                        "AllToAll", ALU.bypass,
                        replica_groups=[list(range(N_CORES))],
                        ins=[a2a_in[h].opt()], outs=[a2a_out[h].opt()],
                        cc_dim="Partition")
                    nc.sync.dma_start(
                        oin[h][:].rearrange("p (s t) -> p s t", t=512),
                        a2a_out[h][:].rearrange("(s p) t -> p s t", p=128))

            # ====================== out projection ====================
            ovs = {h: oin[h][:].rearrange("p (s t) -> p s t", t=512)
                   for h in range(2)}
            with tc.tile_pool(name="opool", bufs=1) as opool:
                def emit_group(ns, split):
                    with tc.tile_pool(name="ops", bufs=1,
                                      space="PSUM") as ops:
                        pso = {}
                        for n in ns:
                            for mp in range(4):
                                pso[(n, mp)] = ops.tile(
                                    [128, 512], F32, tag=f"o{n}{mp}",
                                    name=f"o{n}{mp}")
                        hs_order = ([(0, s) for s in range(8)],
                                    [(1, s) for s in range(8)])
                        if split:
                            passes = [hs_order[0], hs_order[1]]
                        else:
                            passes = [hs_order[0] + hs_order[1]]
                        for pi, hp in enumerate(passes):
                            for n in ns:
                                for mp in range(4):
                                    for ki, (h, s) in enumerate(hp):
                                        first = (pi == 0 and ki == 0)
                                        last = (pi == len(passes) - 1
                                                and ki == len(hp) - 1)
                                        nc.tensor.matmul(
                                            pso[(n, mp)][:],
                                            ovs[h][:, s,
                                                   128 * mp:128 * (mp + 1)],
                                            wov[:, 2 * s + h,
                                                512 * n:512 * (n + 1)],
                                            start=first, stop=last)
                        for n in ns:
                            for mp in range(4):
                                os_t = opool.tile([128, 512], F32,
                                                  tag="osb", bufs=4,
                                                  name="os_t")
                                nc.scalar.copy(os_t[:], pso[(n, mp)][:])
                                nc.sync.dma_start(
                                    out.ap()[128 * mp:128 * (mp + 1),
                                             512 * n:512 * (n + 1)],
                                    os_t[:])

                emit_group([0, 1], split=True)
                emit_group([2, 3], split=False)
            wpool_ctx.__exit__(None, None, None)

    nc.compile()
    return nc


_NC_CACHE = None


def _get_nc():
    global _NC_CACHE
    if _NC_CACHE is None:
        _NC_CACHE = _build()
    return _NC_CACHE


def _host_prep(inputs):
    hs = np.asarray(inputs["hidden_states"], dtype=np.float32)
    Wq = np.asarray(inputs["Wq"], dtype=np.float32)
    Wk = np.asarray(inputs["Wk"], dtype=np.float32)
    Wv = np.asarray(inputs["Wv"], dtype=np.float32)
    Wo = np.asarray(inputs["Wo"], dtype=np.float32)
    cqw = np.asarray(inputs["canon_q_w"], dtype=np.float32)
    ckw = np.asarray(inputs["canon_k_w"], dtype=np.float32)
    cvw = np.asarray(inputs["canon_v_w"], dtype=np.float32)
    qnw = np.asarray(inputs["q_norm_w"], dtype=np.float32)
    knw = np.asarray(inputs["k_norm_w"], dtype=np.float32)

    bf = ml_dtypes.bfloat16
    hsT = np.ascontiguousarray(
        np.concatenate([hs[0].T, hs[1].T], axis=1)).astype(bf)
    WqT, WkT, WvT = Wq.T, Wk.T, Wv.T
    woT = np.ascontiguousarray(Wo.T).astype(bf)

    inv_freq = 1.0 / (10000.0 ** (np.arange(0, DH, 2, dtype=np.float64) / DH))
    freqs = np.arange(S, dtype=np.float64)[:, None] * inv_freq
    emb = np.concatenate([freqs, freqs], axis=-1)
    cosT, sinT = np.cos(emb).T, np.sin(emb).T

    def make_rope(normw, scale):
        A = cosT * normw[:, None] * scale
        wswap = normw[(np.arange(DH) + 64) % DH]
        sign = np.where(np.arange(DH) < 64, -1.0, 1.0)
        Bc = sinT * wswap[:, None] * sign[:, None] * scale
        return (np.ascontiguousarray(A).astype(bf),
                np.ascontiguousarray(Bc).astype(bf))

    Aq, Bq = make_rope(qnw, SCALE)
    Ak, Bk = make_rope(knw, 1.0)

    p = np.arange(128)[:, None]
    f = np.arange(128)[None, :]
    maskd = np.where(p <= f, 0.0, NEG).astype(np.float32)

    in_maps = []
    for r in range(N_CORES):
        wTc = np.ascontiguousarray(np.concatenate(
            [WqT[:, 256 * r:256 * r + 256],
             WkT[:, 128 * r:128 * r + 128],
             WvT[:, 128 * r:128 * r + 128]], axis=1)).astype(bf)
        cwc = np.ascontiguousarray(np.concatenate(
            [cqw[256 * r:256 * r + 256],
             ckw[128 * r:128 * r + 128],
             cvw[128 * r:128 * r + 128]], axis=0)).astype(np.float32)
        in_maps.append({
            "hsT": hsT, "wT": wTc, "woT": woT, "cw": cwc,
            "ropeAq": Aq, "ropeBq": Bq, "ropeAk": Ak, "ropeBk": Bk,
            "maskd": maskd,
        })
    return in_maps


def kernel(**inputs):
    nc = _get_nc()
    in_maps = _host_prep(inputs)
    res = run_bass_kernel_spmd(nc, in_maps, core_ids=list(range(N_CORES)))
    full = np.empty((B, S, D), np.float32)
    for r in range(N_CORES):
        full[r // 4, 512 * (r % 4):512 * (r % 4 + 1), :] = res.results[r]["out"]
    return full


# revision 12
# speedup vs baseline: 1.3323x; 1.1006x over previous
"""Trainium2 Bass kernel for CanonCausalMultiheadAttn.

Sharding: tensor-parallel over heads across 8 cores (2 q-heads + 1 kv-head
per core), both batches replicated. Two head-split AllToAlls exchange
attention outputs so each core owns one (batch, seq-slice) of the final
output projection; the first overlaps the second half of attention and the
second overlaps the h=0 partial pass of the output projection.

Per-core pipeline (shapes hardcoded for B=2, S=2048, D=2048):
  QKV proj (bf16 matmul, weights SBUF-resident) -> canon conv via halo'd
  raw buffer (DVE, bf16) -> qk rmsnorm rstd via PE column-sum matmuls and
  fast-approx reciprocal -> RoPE (DVE bf16; norm-weight & 1/sqrt(dh)
  folded into host tables; q AND k rstd broadcast via K=1 matmuls so the
  attention exp needs no per-partition scale) -> causal attention with
  scores in [Sk, Sq] layout, two Sk-blocks paired per [128,1024] PSUM
  tile so one exp covers both; PV computed transposed (V stationary)
  giving output directly in [dh, Sq]; softmax denominator via ones-column
  matmuls; normalize tail software-pipelined into the next block ->
  AllToAll x2 (head-split) -> output projection in two passes (h=0
  partials to SBUF during the second AllToAll, then h=1 + DVE combine).
"""
import sys

sys.path.insert(0, '/opt/trn_rl_repo')

import numpy as np
import ml_dtypes

import concourse.bass as bass
import concourse.mybir as mybir
import concourse.tile as tile
from concourse import bacc
from concourse.bass_utils import run_bass_kernel_spmd

F32 = mybir.dt.float32
F32R = mybir.dt.float32r
BF16 = mybir.dt.bfloat16
AF = mybir.ActivationFunctionType
ALU = mybir.AluOpType

B, S, D = 2, 2048, 2048
NH, NKV, DH = 16, 8, 128
K_CONV = 4
EPS = 1e-6
SCALE = 1.0 / float(np.sqrt(DH))
NEG = -1e9
N_CORES = 8
NCB = S // 512          # 512-token chunks per batch
N_SKB = S // 128        # Sk blocks per batch


def _build():
    nc = bacc.Bacc("TRN2", target_bir_lowering=False, debug=False,
                   num_devices=N_CORES)

    hsT = nc.dram_tensor("hsT", [D, B * S], BF16, kind="ExternalInput")
    wT = nc.dram_tensor("wT", [D, 512], BF16, kind="ExternalInput")
    woT = nc.dram_tensor("woT", [D, D], BF16, kind="ExternalInput")
    cw = nc.dram_tensor("cw", [512, K_CONV], F32, kind="ExternalInput")
    ropeAq = nc.dram_tensor("ropeAq", [DH, S], BF16, kind="ExternalInput")
    ropeBq = nc.dram_tensor("ropeBq", [DH, S], BF16, kind="ExternalInput")
    ropeAk = nc.dram_tensor("ropeAk", [DH, S], BF16, kind="ExternalInput")
    ropeBk = nc.dram_tensor("ropeBk", [DH, S], BF16, kind="ExternalInput")
    maskd = nc.dram_tensor("maskd", [128, 128], F32, kind="ExternalInput")
    out = nc.dram_tensor("out", [512, D], F32, kind="ExternalOutput")

    with tile.TileContext(nc) as tc:
        with tc.tile_pool(name="const", bufs=1) as cpool, \
             tc.tile_pool(name="persist", bufs=1) as pers, \
             tc.tile_pool(name="dram", bufs=1, space="DRAM") as dram:

            # QKV weights resident in SBUF: [128, 16 k-blocks x 512]
            wT_sb = cpool.tile([128, 16 * 512], BF16, tag="wTsb")
            wv = wT_sb[:].rearrange("p (k c) -> p k c", c=512)
            for kk in range(4):
                nc.sync.dma_start(
                    wv[:, 4 * kk:4 * (kk + 1), :],
                    wT.ap()[512 * kk:512 * (kk + 1), :]
                    .rearrange("(k p) c -> p k c", p=128))

            # ---- constants ----
            ropes = {}
            for nm, t in (("Aq", ropeAq), ("Bq", ropeBq),
                          ("Ak", ropeAk), ("Bk", ropeBk)):
                rt = cpool.tile([DH, S], BF16, tag=f"rope{nm}",
                                name=f"rope{nm}")
                nc.sync.dma_start(rt[:], t.ap())
                ropes[nm] = rt
            mask_sb = cpool.tile([128, 128], F32, tag="mask")
            nc.sync.dma_start(mask_sb[:], maskd.ap())
            cw_sb = []
            for mt in range(4):
                t = cpool.tile([128, K_CONV], F32, tag=f"cw{mt}",
                               name=f"cw{mt}")
                nc.sync.dma_start(t[:], cw.ap()[128 * mt:128 * mt + 128, :])
                cw_sb.append(t)
            ones_col = cpool.tile([128, 1], BF16, tag="oc")
            nc.vector.memset(ones_col[:], 1.0)
            eps_sb = cpool.tile([1, 1], F32, tag="eps")
            nc.vector.memset(eps_sb[:], EPS)
            ones_row = cpool.tile([1, 128], F32, tag="or")
            nc.vector.memset(ones_row[:], 1.0)
            s0_sb = []
            for mt in range(4):
                t = cpool.tile([128, 1], F32, tag=f"s0{mt}", name=f"s0{mt}")
                nc.vector.tensor_scalar_add(t[:], cw_sb[mt][:, 0:1], 1.0)
                s0_sb.append(t)

            # persistent per-(b,mt) tiles
            roped = {}   # (b, mt) -> [128, S] bf16 (rstd folded in)
            vT = {}      # b -> [128, N_SKB*128] bf16 (V transposed blocks)

            for b in range(B):
                vT[b] = pers.tile([128, N_SKB * 128], BF16, tag=f"vT{b}",
                                  name=f"vT{b}")
                for mt in range(3):
                    roped[(b, mt)] = pers.tile(
                        [128, S], BF16, tag=f"roped{b}{mt}",
                        name=f"roped{b}{mt}")

            # ============ phase Q: QKV + canon + norm + rope ============
            with tc.tile_pool(name="qps", bufs=1, space="PSUM") as qps, \
                 tc.tile_pool(name="spp", bufs=2, space="PSUM") as spp, \
                 tc.tile_pool(name="bps", bufs=2, space="PSUM") as bps:
                for b in range(B):
                    with tc.tile_pool(name=f"bwork{b}", bufs=1) as bw:
                        rn_sb = {}   # mt -> [1, S] f32 rstd rows (batch-local)
                        for mt in range(3):
                            rn_sb[(b, mt)] = bw.tile(
                                [1, S], F32, tag=f"rn{mt}", name=f"rn{mt}")
                        cn = {}
                        raw_h = {}
                        for mt in range(4):
                            cn[mt] = bw.tile([128, S], BF16, tag=f"cn{mt}",
                                             name=f"cn{mt}")
                            raw_h[mt] = bw.tile([128, 516], BF16,
                                                tag=f"rawh{mt}",
                                                name=f"raw_h{mt}")
                            nc.vector.memset(raw_h[mt][:, 0:4], 0.0)
                        psums = [qps.tile([128, 512], F32, tag=f"qk{mt}",
                                          name=f"qk{mt}") for mt in range(4)]

                        def emit_chunk_mms(n):
                            hs_sb = bw.tile([128, 16 * 512], BF16,
                                            tag="hschunk", bufs=2,
                                            name="hs_sb")
                            nc.sync.dma_start(
                                hs_sb[:].rearrange("p (k s) -> p k s", s=512),
                                hsT.ap()[:, b * S + 512 * n:
                                         b * S + 512 * (n + 1)]
                                .rearrange("(k p) s -> p k s", p=128))
                            hvv = hs_sb[:].rearrange("p (k s) -> p k s",
                                                     s=512)
                            for k in range(16):
                                for mt in range(4):
                                    nc.tensor.matmul(
                                        psums[mt][:],
                                        wv[:, k, 128 * mt:128 * (mt + 1)],
                                        hvv[:, k, :],
                                        start=(k == 0), stop=(k == 15))

                        def emit_canon(n):
                            lo = 512 * n
                            for mt in range(4):
                                rh = raw_h[mt]
                                if n > 0:
                                    nc.vector.tensor_copy(
                                        rh[:, 1:4], rh[:, 513:516])
                                nc.scalar.copy(rh[:, 4:516], psums[mt][:])
                                c = cn[mt]
                                nc.vector.tensor_scalar_mul(
                                    c[:, lo:lo + 512], rh[:, 4:516],
                                    s0_sb[mt][:])
                                for k in range(1, K_CONV):
                                    nc.vector.scalar_tensor_tensor(
                                        c[:, lo:lo + 512],
                                        rh[:, 4 - k:516 - k],
                                        cw_sb[mt][:, k:k + 1],
                                        c[:, lo:lo + 512],
                                        ALU.mult, ALU.add)
                            # squares for rmsnorm (q0, q1, k)
                            for mt in range(3):
                                sq = bw.tile([128, 512], BF16, tag="sqr",
                                             bufs=3, name="sq")
                                nc.vector.tensor_mul(
                                    sq[:], cn[mt][:, lo:lo + 512],
                                    cn[mt][:, lo:lo + 512])
                                sqs[(n, mt)] = sq

                        def emit_norm(n):
                            for mt in range(3):
                                sp = spp.tile([1, 512], F32, tag="ssq")
                                nc.tensor.matmul(sp[:], ones_col[:],
                                                 sqs.pop((n, mt))[:],
                                                 start=True, stop=True)
                                srt = bw.tile([1, 512], F32, tag="srt",
                                              bufs=2, name="srt")
                                nc.scalar.activation(srt[:], sp[:], AF.Sqrt,
                                                     bias=eps_sb[:],
                                                     scale=1.0 / DH)
                                nc.vector.reciprocal_approx_fast(
                                    rn_sb[(b, mt)][:, 512 * n:512 * (n + 1)],
                                    srt[:])
                            # V transpose blocks for this chunk
                            for t in range(4):
                                i = 4 * n + t
                                nc.sync.dma_start_transpose(
                                    vT[b][:, 128 * i:128 * (i + 1)],
                                    cn[3][:, 128 * i:128 * (i + 1)])

                        sqs = {}
                        for n in range(NCB + 1):
                            if n < NCB:
                                emit_chunk_mms(n)
                            if n >= 1:
                                emit_norm(n - 1)
                            if n < NCB:
                                emit_canon(n)

                        # rope with rstd folded in; k (mt=2) first so the
                        # attention KT dependency clears earliest
                        for mt in (2, 0, 1):
                            is_q = mt < 2
                            x = cn[mt]
                            A_ = ropes["Aq"] if is_q else ropes["Ak"]
                            B_ = ropes["Bq"] if is_q else ropes["Bk"]
                            sh = bw.tile([128, S], BF16, tag="shift",
                                         bufs=2, name="sh")
                            nc.sync.dma_start(sh[0:64, :], x[64:128, :])
                            nc.sync.dma_start(sh[64:128, :], x[0:64, :])
                            nc.vector.tensor_mul(sh[:], sh[:], B_[:])
                            tm = bw.tile([128, S], BF16, tag="ropetmp",
                                         bufs=2, name="tm")
                            nc.vector.tensor_mul(tm[:], x[:], A_[:])
                            nc.vector.tensor_add(tm[:], tm[:], sh[:])
                            ro = roped[(b, mt)]
                            for c in range(NCB):
                                bp = bps.tile([128, 512], F32, tag="bcp")
                                nc.tensor.matmul(
                                    bp[:], ones_row[:],
                                    rn_sb[(b, mt)][:, 512 * c:512 * (c + 1)],
                                    start=True, stop=True)
                                bcb = bw.tile([128, 512], BF16, tag="bcb",
                                              bufs=2, name="bcb")
                                nc.scalar.copy(bcb[:], bp[:])
                                nc.vector.tensor_mul(
                                    ro[:, 512 * c:512 * (c + 1)],
                                    tm[:, 512 * c:512 * (c + 1)],
                                    bcb[:])

            # ============ attention + head-split all-to-all ============
            wpool_ctx = tc.tile_pool(name="wpool", bufs=1)
            wpool = wpool_ctx.__enter__()
            # Wo resident prefetch (needed only for the output projection)
            wo_sb = wpool.tile([128, 16 * D], BF16, tag="wosb")
            wov = wo_sb[:].rearrange("p (g o) -> p g o", o=D)
            for gg in range(4):
                nc.sync.dma_start(
                    wov[:, 4 * gg:4 * (gg + 1), :],
                    woT.ap()[512 * gg:512 * (gg + 1), :]
                    .rearrange("(g p) o -> p g o", p=128))

            a2a_in = {}
            a2a_out = {}
            oin = {}
            for h in range(2):
                a2a_in[h] = dram.tile([1024, 512], BF16, tag=f"a2ain{h}",
                                      name=f"a2a_in{h}")
                a2a_out[h] = dram.tile([1024, 512], BF16, tag=f"a2aout{h}",
                                       name=f"a2a_out{h}")
                oin[h] = wpool.tile([128, 8 * 512], BF16, tag=f"oin{h}",
                                    name=f"oin{h}")

            with tc.tile_pool(name="scps", bufs=2, space="PSUM") as scps, \
                 tc.tile_pool(name="pvps", bufs=2, space="PSUM") as pvps, \
                 tc.tile_pool(name="dnps", bufs=1, space="PSUM") as dnps, \
                 tc.tile_pool(name="bcps", bufs=1, space="PSUM") as bcps, \
                 tc.tile_pool(name="apool", bufs=1) as apool:
                pending = [None]

                def flush_tail():
                    if pending[0] is None:
                        return
                    pv, dn, h, b, j = pending[0]
                    pending[0] = None
                    rec = apool.tile([1, 512], F32, tag="rec",
                                     bufs=2, name="rec")
                    nc.vector.reciprocal_approx_fast(rec[:], dn[:])
                    bc = bcps.tile([128, 512], F32, tag="bc", name="bc")
                    nc.tensor.matmul(bc[:], ones_row[:], rec[:],
                                     start=True, stop=True)
                    bcb = apool.tile([128, 512], BF16, tag="bcbn",
                                     bufs=2, name="bcb")
                    nc.vector.tensor_copy(bcb[:], bc[:])
                    nrm = apool.tile([128, 512], BF16, tag="nrm",
                                     bufs=2, name="nrm")
                    nc.vector.tensor_mul(nrm[:], pv[:], bcb[:])
                    nc.sync.dma_start(
                        a2a_in[h][128 * (4 * b + j):
                                  128 * (4 * b + j + 1), :],
                        nrm[:])

                for h in range(2):
                    for b in range(B):
                        KT = roped[(b, 2)]
                        QT = roped[(b, h)]
                        vt = vT[b]
                        for j in range(NCB):
                            pv = pvps.tile([128, 512], F32, tag="pv",
                                           name="pv")
                            dn = dnps.tile([1, 512], F32, tag="dn",
                                           name="dn")
                            nprs = 2 * j + 2   # pairs of Sk blocks
                            pts = [None] * nprs

                            def emit_qk(pr):
                                ps = scps.tile([128, 1024], F32, tag="sc",
                                               name="ps")
                                pt = apool.tile([128, 1024], BF16, tag="p",
                                                bufs=4, name="pt")
                                offs = []
                                for half in range(2):
                                    i = 2 * pr + half
                                    r = i - 4 * j
                                    off = 128 * max(r, 0)
                                    offs.append(off)
                                    base = 512 * half
                                    nc.tensor.matmul(
                                        ps[:, base + off:base + 512],
                                        KT[:, 128 * i:128 * (i + 1)],
                                        QT[:, 512 * j + off:512 * (j + 1)],
                                        start=True, stop=True)
                                    if r >= 0:
                                        nc.vector.tensor_add(
                                            ps[:, base + off:base + off + 128],
                                            ps[:, base + off:base + off + 128],
                                            mask_sb[:])
                                if offs[0] > 0:
                                    nc.vector.memset(pt[:, 0:offs[0]], 0.0)
                                if offs[1] > 0:
                                    nc.vector.memset(
                                        ps[:, 512:512 + offs[1]], NEG)
                                nc.scalar.activation(
                                    pt[:, offs[0]:1024],
                                    ps[:, offs[0]:1024], AF.Exp)
                                pts[pr] = pt

                            def emit_pv(pr):
                                pt = pts[pr]
                                for half in range(2):
                                    i = 2 * pr + half
                                    first = (i == 0)
                                    last = (i == 4 * j + 3)
                                    base = 512 * half
                                    nc.tensor.matmul(
                                        pv[:],
                                        vt[:, 128 * i:128 * (i + 1)],
                                        pt[:, base:base + 512],
                                        start=first, stop=last)
                                    nc.tensor.matmul(
                                        dn[:], ones_col[:],
                                        pt[:, base:base + 512],
                                        start=first, stop=last)

                            for pr in range(nprs):
                                emit_qk(pr)
                                if pr == 0:
                                    flush_tail()
                                if pr >= 1:
                                    emit_pv(pr - 1)
                            emit_pv(nprs - 1)
                            pending[0] = (pv, dn, h, b, j)
                    flush_tail()
                    nc.gpsimd.collective_compute(
                        "AllToAll", ALU.bypass,
                        replica_groups=[list(range(N_CORES))],
                        ins=[a2a_in[h].opt()], outs=[a2a_out[h].opt()],
                        cc_dim="Partition")
                    nc.sync.dma_start(
                        oin[h][:].rearrange("p (s t) -> p s t", t=512),
                        a2a_out[h][:].rearrange("(s p) t -> p s t", p=128))

            # ====================== out projection ====================
            # pass A: h=0 partial sums for all (n, mp) -> SBUF (runs during
            # the second all-to-all); pass B: h=1 partials + DVE combine.
            ovs = {h: oin[h][:].rearrange("p (s t) -> p s t", t=512)
                   for h in range(2)}
            with tc.tile_pool(name="opool", bufs=1) as opool, \
                 tc.tile_pool(name="ops", bufs=2, space="PSUM") as ops:
                ph0 = {}
                for n in range(4):
                    for mp in range(4):
                        pso = ops.tile([128, 512], F32, tag=f"oa{mp}",
                                       name=f"oa{mp}")
                        for s in range(8):
                            nc.tensor.matmul(
                                pso[:],
                                ovs[0][:, s, 128 * mp:128 * (mp + 1)],
                                wov[:, 2 * s, 512 * n:512 * (n + 1)],
                                start=(s == 0), stop=(s == 7))
                        pt0 = opool.tile([128, 512], F32, tag="ph0",
                                         bufs=16, name="pt0")
                        nc.scalar.copy(pt0[:], pso[:])
                        ph0[(n, mp)] = pt0
                for n in range(4):
                    for mp in range(4):
                        pso = ops.tile([128, 512], F32, tag=f"oa{mp}",
                                       name=f"ob{mp}")
                        for s in range(8):
                            nc.tensor.matmul(
                                pso[:],
                                ovs[1][:, s, 128 * mp:128 * (mp + 1)],
                                wov[:, 2 * s + 1, 512 * n:512 * (n + 1)],
                                start=(s == 0), stop=(s == 7))
                        os_t = opool.tile([128, 512], F32, tag="osb",
                                          bufs=4, name="os_t")
                        nc.vector.tensor_add(os_t[:], pso[:],
                                             ph0[(n, mp)][:])
                        nc.sync.dma_start(
                            out.ap()[128 * mp:128 * (mp + 1),
                                     512 * n:512 * (n + 1)],
                            os_t[:])
            wpool_ctx.__exit__(None, None, None)

    nc.compile()
    return nc


_NC_CACHE = None


def _get_nc():
    global _NC_CACHE
    if _NC_CACHE is None:
        _NC_CACHE = _build()
    return _NC_CACHE


def _host_prep(inputs):
    hs = np.asarray(inputs["hidden_states"], dtype=np.float32)
    Wq = np.asarray(inputs["Wq"], dtype=np.float32)
    Wk = np.asarray(inputs["Wk"], dtype=np.float32)
    Wv = np.asarray(inputs["Wv"], dtype=np.float32)
    Wo = np.asarray(inputs["Wo"], dtype=np.float32)
    cqw = np.asarray(inputs["canon_q_w"], dtype=np.float32)
    ckw = np.asarray(inputs["canon_k_w"], dtype=np.float32)
    cvw = np.asarray(inputs["canon_v_w"], dtype=np.float32)
    qnw = np.asarray(inputs["q_norm_w"], dtype=np.float32)
    knw = np.asarray(inputs["k_norm_w"], dtype=np.float32)

    bf = ml_dtypes.bfloat16
    hsT = np.ascontiguousarray(
        np.concatenate([hs[0].T, hs[1].T], axis=1)).astype(bf)
    WqT, WkT, WvT = Wq.T, Wk.T, Wv.T
    woT = np.ascontiguousarray(Wo.T).astype(bf)

    inv_freq = 1.0 / (10000.0 ** (np.arange(0, DH, 2, dtype=np.float64) / DH))
    freqs = np.arange(S, dtype=np.float64)[:, None] * inv_freq
    emb = np.concatenate([freqs, freqs], axis=-1)
    cosT, sinT = np.cos(emb).T, np.sin(emb).T

    def make_rope(normw, scale):
        A = cosT * normw[:, None] * scale
        wswap = normw[(np.arange(DH) + 64) % DH]
        sign = np.where(np.arange(DH) < 64, -1.0, 1.0)
        Bc = sinT * wswap[:, None] * sign[:, None] * scale
        return (np.ascontiguousarray(A).astype(bf),
                np.ascontiguousarray(Bc).astype(bf))

    Aq, Bq = make_rope(qnw, SCALE)
    Ak, Bk = make_rope(knw, 1.0)

    p = np.arange(128)[:, None]
    f = np.arange(128)[None, :]
    maskd = np.where(p <= f, 0.0, NEG).astype(np.float32)

    in_maps = []
    for r in range(N_CORES):
        wTc = np.ascontiguousarray(np.concatenate(
            [WqT[:, 256 * r:256 * r + 256],
             WkT[:, 128 * r:128 * r + 128],
             WvT[:, 128 * r:128 * r + 128]], axis=1)).astype(bf)
        cwc = np.ascontiguousarray(np.concatenate(
            [cqw[256 * r:256 * r + 256],
             ckw[128 * r:128 * r + 128],
             cvw[128 * r:128 * r + 128]], axis=0)).astype(np.float32)
        in_maps.append({
            "hsT": hsT, "wT": wTc, "woT": woT, "cw": cwc,
            "ropeAq": Aq, "ropeBq": Bq, "ropeAk": Ak, "ropeBk": Bk,
            "maskd": maskd,
        })
    return in_maps


def kernel(**inputs):
    nc = _get_nc()
    in_maps = _host_prep(inputs)
    res = run_bass_kernel_spmd(nc, in_maps, core_ids=list(range(N_CORES)))
    full = np.empty((B, S, D), np.float32)
    for r in range(N_CORES):
        full[r // 4, 512 * (r % 4):512 * (r % 4 + 1), :] = res.results[r]["out"]
    return full


# revision 19
# speedup vs baseline: 1.4357x; 1.0776x over previous
"""Trainium2 Bass kernel for CanonCausalMultiheadAttn.

Sharding: tensor-parallel over heads across 8 cores (2 q-heads + 1 kv-head
per core), both batches replicated. Two head-split AllToAlls exchange
attention outputs so each core owns one (batch, seq-slice) of the final
output projection; the first overlaps the second half of attention and the
second overlaps the h=0 partial pass of the output projection.

Per-core pipeline (shapes hardcoded for B=2, S=2048, D=2048):
  QKV proj (bf16 matmul, weights SBUF-resident) -> canon conv via halo'd
  raw buffer (DVE, bf16) -> qk rmsnorm rstd via PE column-sum matmuls and
  fast-approx reciprocal -> RoPE (DVE bf16; norm-weight & 1/sqrt(dh)
  folded into host tables; q AND k rstd folded in via K=1 broadcast
  matmuls whose emission is software-pipelined into later PE-dense
  phases) -> causal attention with scores in [Sk, Sq] layout, two
  Sk-blocks paired per [128,1024] PSUM tile so one exp covers both; the
  causal mask and the fully-masked wedge are added on the PE itself
  (maskT.T @ I accumulation) so exp depends only on the PE; PV computed
  transposed (V stationary) directly in [dh, Sq]; softmax denominator via
  ones-column matmuls; normalize tail software-pipelined into the next
  block -> AllToAll x2 (head-split) -> output projection in two passes
  (h=0 partials to SBUF during the second AllToAll, then h=1 + combine).
"""
import sys

sys.path.insert(0, '/opt/trn_rl_repo')

import numpy as np
import ml_dtypes

import concourse.bass as bass
import concourse.mybir as mybir
import concourse.tile as tile
from concourse import bacc
from concourse.bass_utils import run_bass_kernel_spmd

F32 = mybir.dt.float32
F32R = mybir.dt.float32r
BF16 = mybir.dt.bfloat16
AF = mybir.ActivationFunctionType
ALU = mybir.AluOpType

B, S, D = 2, 2048, 2048
NH, NKV, DH = 16, 8, 128
K_CONV = 4
EPS = 1e-6
SCALE = 1.0 / float(np.sqrt(DH))
NEG = -1e9
N_CORES = 8
NCB = S // 512          # 512-token chunks per batch
N_SKB = S // 128        # Sk blocks per batch


def _build():
    nc = bacc.Bacc("TRN2", target_bir_lowering=False, debug=False,
                   num_devices=N_CORES)

    hsT = nc.dram_tensor("hsT", [D, B * S], BF16, kind="ExternalInput")
    wT = nc.dram_tensor("wT", [D, 512], BF16, kind="ExternalInput")
    woT = nc.dram_tensor("woT", [D, D], BF16, kind="ExternalInput")
    cw = nc.dram_tensor("cw", [512, K_CONV], F32, kind="ExternalInput")
    ropeAq = nc.dram_tensor("ropeAq", [DH, S], BF16, kind="ExternalInput")
    ropeBq = nc.dram_tensor("ropeBq", [DH, S], BF16, kind="ExternalInput")
    ropeAk = nc.dram_tensor("ropeAk", [DH, S], BF16, kind="ExternalInput")
    ropeBk = nc.dram_tensor("ropeBk", [DH, S], BF16, kind="ExternalInput")
    maskTb = nc.dram_tensor("maskTb", [128, 128], BF16, kind="ExternalInput")
    idb = nc.dram_tensor("idb", [128, 128], BF16, kind="ExternalInput")
    out = nc.dram_tensor("out", [512, D], F32, kind="ExternalOutput")

    with tile.TileContext(nc) as tc:
        with tc.tile_pool(name="const", bufs=1) as cpool, \
             tc.tile_pool(name="persist", bufs=1) as pers, \
             tc.tile_pool(name="dram", bufs=1, space="DRAM") as dram:

            # QKV weights resident in SBUF: [128, 16 k-blocks x 512]
            wT_sb = cpool.tile([128, 16 * 512], BF16, tag="wTsb")
            wv = wT_sb[:].rearrange("p (k c) -> p k c", c=512)
            for kk in range(4):
                nc.sync.dma_start(
                    wv[:, 4 * kk:4 * (kk + 1), :],
                    wT.ap()[512 * kk:512 * (kk + 1), :]
                    .rearrange("(k p) c -> p k c", p=128))

            # ---- constants (rope tables last: not needed until late) ----
            mask_sb = cpool.tile([128, 128], BF16, tag="mask")
            nc.sync.dma_start(mask_sb[:], maskTb.ap())
            id_sb = cpool.tile([128, 128], BF16, tag="idsb")
            nc.sync.dma_start(id_sb[:], idb.ap())
            cw_sb = []
            for mt in range(4):
                t = cpool.tile([128, K_CONV], F32, tag=f"cw{mt}",
                               name=f"cw{mt}")
                nc.sync.dma_start(t[:], cw.ap()[128 * mt:128 * mt + 128, :])
                cw_sb.append(t)
            ones_col = cpool.tile([128, 1], BF16, tag="oc")
            nc.vector.memset(ones_col[:], 1.0)
            eps_sb = cpool.tile([1, 1], F32, tag="eps")
            nc.vector.memset(eps_sb[:], EPS)
            ones_row = cpool.tile([1, 128], F32, tag="or")
            nc.vector.memset(ones_row[:], 1.0)
            # NEG broadcast: negrow.T @ onesb[:, :w] == NEG everywhere
            negrow = cpool.tile([128, 128], BF16, tag="negrow")
            nc.vector.memset(negrow[:], 0.0)
            nc.vector.memset(negrow[0:1, :], NEG)
            onesb = cpool.tile([128, 512], BF16, tag="onesb")
            nc.vector.memset(onesb[:], 1.0)
            s0_sb = []
            for mt in range(4):
                t = cpool.tile([128, 1], F32, tag=f"s0{mt}", name=f"s0{mt}")
                nc.vector.tensor_scalar_add(t[:], cw_sb[mt][:, 0:1], 1.0)
                s0_sb.append(t)
            ropes = {}
            for nm, t in (("Aq", ropeAq), ("Bq", ropeBq),
                          ("Ak", ropeAk), ("Bk", ropeBk)):
                rt = cpool.tile([DH, S], BF16, tag=f"rope{nm}",
                                name=f"rope{nm}")
                nc.sync.dma_start(rt[:], t.ap())
                ropes[nm] = rt

            # persistent per-(b,mt) tiles
            roped = {}   # (b, mt) -> [128, S] bf16 (rstd folded in)
            vT = {}      # b -> [128, N_SKB*128] bf16 (V transposed blocks)

            for b in range(B):
                vT[b] = pers.tile([128, N_SKB * 128], BF16, tag=f"vT{b}",
                                  name=f"vT{b}")
                for mt in range(3):
                    roped[(b, mt)] = pers.tile(
                        [128, S], BF16, tag=f"roped{b}{mt}",
                        name=f"roped{b}{mt}")

            # rstd broadcast tiles are precomputed during the PE-dense chunk
            # pipeline; the deferred rope stage-2 is then pure DVE in-place
            # multiplies (no PE involvement), dripped into later phases.
            bcall_ctx = tc.tile_pool(name="bcall", bufs=1)
            bcall = bcall_ctx.__enter__()
            bcb_store = {}
            stage2 = {0: [], 1: []}

            # ============ phase Q: QKV + canon + norm + rope ============
            with tc.tile_pool(name="qps", bufs=1, space="PSUM") as qps, \
                 tc.tile_pool(name="spp", bufs=2, space="PSUM") as spp, \
                 tc.tile_pool(name="bps", bufs=2, space="PSUM") as bps:
                for b in range(B):
                    with tc.tile_pool(name=f"bwork{b}", bufs=1) as bw:
                        rn_sb = {}   # mt -> [1, S] f32 rstd rows
                        for mt in range(3):
                            rn_sb[mt] = bw.tile(
                                [1, S], F32, tag=f"rn{mt}", name=f"rn{mt}")
                        cn = {}
                        raw_h = {}
                        for mt in range(4):
                            cn[mt] = bw.tile([128, S], BF16, tag=f"cn{mt}",
                                             name=f"cn{mt}")
                            raw_h[mt] = bw.tile([128, 516], BF16,
                                                tag=f"rawh{mt}",
                                                name=f"raw_h{mt}")
                            nc.vector.memset(raw_h[mt][:, 0:4], 0.0)
                        psums = [qps.tile([128, 512], F32, tag=f"qk{mt}",
                                          name=f"qk{mt}") for mt in range(4)]

                        def emit_chunk_mms(n):
                            hs_sb = bw.tile([128, 16 * 512], BF16,
                                            tag="hschunk", bufs=2,
                                            name="hs_sb")
                            nc.sync.dma_start(
                                hs_sb[:].rearrange("p (k s) -> p k s", s=512),
                                hsT.ap()[:, b * S + 512 * n:
                                         b * S + 512 * (n + 1)]
                                .rearrange("(k p) s -> p k s", p=128))
                            hvv = hs_sb[:].rearrange("p (k s) -> p k s",
                                                     s=512)
                            for k in range(16):
                                for mt in range(4):
                                    nc.tensor.matmul(
                                        psums[mt][:],
                                        wv[:, k, 128 * mt:128 * (mt + 1)],
                                        hvv[:, k, :],
                                        start=(k == 0), stop=(k == 15))

                        def emit_canon(n):
                            lo = 512 * n
                            for mt in range(4):
                                rh = raw_h[mt]
                                if n > 0:
                                    nc.vector.tensor_copy(
                                        rh[:, 1:4], rh[:, 513:516])
                                nc.scalar.copy(rh[:, 4:516], psums[mt][:])
                                c = cn[mt]
                                nc.vector.tensor_scalar_mul(
                                    c[:, lo:lo + 512], rh[:, 4:516],
                                    s0_sb[mt][:])
                                for k in range(1, K_CONV):
                                    nc.vector.scalar_tensor_tensor(
                                        c[:, lo:lo + 512],
                                        rh[:, 4 - k:516 - k],
                                        cw_sb[mt][:, k:k + 1],
                                        c[:, lo:lo + 512],
                                        ALU.mult, ALU.add)
                            # squares for rmsnorm (q0, q1, k)
                            for mt in range(3):
                                sq = bw.tile([128, 512], BF16, tag="sqr",
                                             bufs=3, name="sq")
                                nc.vector.tensor_mul(
                                    sq[:], cn[mt][:, lo:lo + 512],
                                    cn[mt][:, lo:lo + 512])
                                sqs[(n, mt)] = sq

                        def emit_norm(n):
                            for mt in range(3):
                                sp = spp.tile([1, 512], F32, tag="ssq")
                                nc.tensor.matmul(sp[:], ones_col[:],
                                                 sqs.pop((n, mt))[:],
                                                 start=True, stop=True)
                                srt = bw.tile([1, 512], F32, tag="srt",
                                              bufs=2, name="srt")
                                nc.scalar.activation(srt[:], sp[:], AF.Sqrt,
                                                     bias=eps_sb[:],
                                                     scale=1.0 / DH)
                                nc.vector.reciprocal_approx_fast(
                                    rn_sb[mt][:, 512 * n:512 * (n + 1)],
                                    srt[:])
                            # V transpose blocks for this chunk
                            for t in range(4):
                                i = 4 * n + t
                                nc.sync.dma_start_transpose(
                                    vT[b][:, 128 * i:128 * (i + 1)],
                                    cn[3][:, 128 * i:128 * (i + 1)])

                        def emit_bc(n):
                            # rstd broadcast for chunk n (PE + ACT only;
                            # emitted inside the PE-dense chunk stream)
                            for mt in range(3):
                                bp = bps.tile([128, 512], F32, tag="bcp")
                                nc.tensor.matmul(
                                    bp[:], ones_row[:],
                                    rn_sb[mt][:, 512 * n:512 * (n + 1)],
                                    start=True, stop=True)
                                bcb = bcall.tile([128, 512], BF16,
                                                 tag=f"bcs{mt}{n}",
                                                 name=f"bcs{mt}{n}")
                                nc.scalar.copy(bcb[:], bp[:])
                                bcb_store[(b, mt, n)] = bcb

                        def flush_stage2(lst, k):
                            for _ in range(k):
                                if lst:
                                    lst.pop(0)()

                        sqs = {}
                        for n in range(NCB + 2):
                            if n < NCB:
                                emit_chunk_mms(n)
                            if b == 1:
                                flush_stage2(stage2[0], 3)
                            if 1 <= n <= NCB:
                                emit_norm(n - 1)
                            if n >= 2:
                                emit_bc(n - 2)
                            if n < NCB:
                                emit_canon(n)

                        # rope stage 1: the big DVE elementwise chain,
                        # written directly into roped (k first so the
                        # attention KT dependency clears earliest);
                        # stage 2 = in-place rstd scaling, deferred
                        for mt in (2, 0, 1):
                            is_q = mt < 2
                            x = cn[mt]
                            A_ = ropes["Aq"] if is_q else ropes["Ak"]
                            B_ = ropes["Bq"] if is_q else ropes["Bk"]
                            sh = bw.tile([128, S], BF16, tag="shift",
                                         bufs=2, name="sh")
                            nc.sync.dma_start(sh[0:64, :], x[64:128, :])
                            nc.sync.dma_start(sh[64:128, :], x[0:64, :])
                            nc.vector.tensor_mul(sh[:], sh[:], B_[:])
                            ro = roped[(b, mt)]
                            nc.vector.tensor_mul(ro[:], x[:], A_[:])
                            nc.vector.tensor_add(ro[:], ro[:], sh[:])

                        def make_s2(b_, mt_, c_):
                            def emit():
                                nc.vector.tensor_mul(
                                    roped[(b_, mt_)]
                                    [:, 512 * c_:512 * (c_ + 1)],
                                    roped[(b_, mt_)]
                                    [:, 512 * c_:512 * (c_ + 1)],
                                    bcb_store[(b_, mt_, c_)][:])
                            return emit

                        for mt in (2, 0, 1):
                            for c in range(NCB):
                                stage2[b].append(make_s2(b, mt, c))

                # end of b loop: flush any stage2[0] leftovers
                while stage2[0]:
                    stage2[0].pop(0)()

            # ============ attention + head-split all-to-all ============
            wpool_ctx = tc.tile_pool(name="wpool", bufs=1)
            wpool = wpool_ctx.__enter__()
            # Wo resident prefetch (needed only for the output projection)
            wo_sb = wpool.tile([128, 16 * D], BF16, tag="wosb")
            wov = wo_sb[:].rearrange("p (g o) -> p g o", o=D)
            for gg in range(4):
                nc.sync.dma_start(
                    wov[:, 4 * gg:4 * (gg + 1), :],
                    woT.ap()[512 * gg:512 * (gg + 1), :]
                    .rearrange("(g p) o -> p g o", p=128))

            a2a_in = {}
            a2a_out = {}
            oin = {}
            for h in range(2):
                a2a_in[h] = dram.tile([1024, 512], BF16, tag=f"a2ain{h}",
                                      name=f"a2a_in{h}")
                a2a_out[h] = dram.tile([1024, 512], BF16, tag=f"a2aout{h}",
                                       name=f"a2a_out{h}")
                oin[h] = wpool.tile([128, 8 * 512], BF16, tag=f"oin{h}",
                                    name=f"oin{h}")

            with tc.tile_pool(name="scps", bufs=2, space="PSUM") as scps, \
                 tc.tile_pool(name="pvps", bufs=2, space="PSUM") as pvps, \
                 tc.tile_pool(name="dnps", bufs=1, space="PSUM") as dnps, \
                 tc.tile_pool(name="bcps", bufs=1, space="PSUM") as bcps, \
                 tc.tile_pool(name="apool", bufs=1) as apool:
                pending = [None]

                def flush_tail():
                    if pending[0] is None:
                        return
                    pv, dn, h, b, j = pending[0]
                    pending[0] = None
                    rec = apool.tile([1, 512], F32, tag="rec",
                                     bufs=2, name="rec")
                    nc.vector.reciprocal_approx_fast(rec[:], dn[:])
                    bc = bcps.tile([128, 512], F32, tag="bc", name="bc")
                    nc.tensor.matmul(bc[:], ones_row[:], rec[:],
                                     start=True, stop=True)
                    bcb = apool.tile([128, 512], BF16, tag="bcbn",
                                     bufs=2, name="bcb")
                    nc.vector.tensor_copy(bcb[:], bc[:])
                    nrm = apool.tile([128, 512], BF16, tag="nrm",
                                     bufs=2, name="nrm")
                    nc.vector.tensor_mul(nrm[:], pv[:], bcb[:])
                    nc.sync.dma_start(
                        a2a_in[h][128 * (4 * b + j):
                                  128 * (4 * b + j + 1), :],
                        nrm[:])

                for h in range(2):
                    for b in range(B):
                        KT = roped[(b, 2)]
                        QT = roped[(b, h)]
                        vt = vT[b]
                        for j in range(NCB):
                            pv = pvps.tile([128, 512], F32, tag="pv",
                                           name="pv")
                            dn = dnps.tile([1, 512], F32, tag="dn",
                                           name="dn")
                            nprs = 2 * j + 2   # pairs of Sk blocks
                            pts = [None] * nprs
                            offp = [None] * nprs

                            def emit_qk(pr):
                                ps = scps.tile([128, 1024], F32, tag="sc",
                                               name="ps")
                                pt = apool.tile([128, 1024], BF16, tag="p",
                                                bufs=4, name="pt")
                                offs = []
                                for half in range(2):
                                    i = 2 * pr + half
                                    r = i - 4 * j
                                    off = 128 * max(r, 0)
                                    offs.append(off)
                                    base = 512 * half
                                    diag = (r >= 0)
                                    nc.tensor.matmul(
                                        ps[:, base + off:base + 512],
                                        KT[:, 128 * i:128 * (i + 1)],
                                        QT[:, 512 * j + off:512 * (j + 1)],
                                        start=True, stop=not diag)
                                    if diag:
                                        nc.tensor.matmul(
                                            ps[:, base + off:
                                               base + off + 128],
                                            mask_sb[:], id_sb[:],
                                            start=False, stop=True,
                                            skip_group_check=True)
                                if offs[1] > 0:
                                    nc.tensor.matmul(
                                        ps[:, 512:512 + offs[1]],
                                        negrow[:], onesb[:, 0:offs[1]],
                                        start=True, stop=True)
                                nc.scalar.activation(
                                    pt[:, offs[0]:1024],
                                    ps[:, offs[0]:1024], AF.Exp)
                                pts[pr] = pt
                                offp[pr] = offs

                            def emit_pv(pr):
                                pt = pts[pr]
                                offs = offp[pr]
                                for half in range(2):
                                    i = 2 * pr + half
                                    off = offs[half]
                                    first = (i == 0)
                                    last = (i == 4 * j + 3)
                                    base = 512 * half
                                    nc.tensor.matmul(
                                        pv[:, off:512],
                                        vt[:, 128 * i:128 * (i + 1)],
                                        pt[:, base + off:base + 512],
                                        start=first, stop=last,
                                        skip_group_check=True)
                                    nc.tensor.matmul(
                                        dn[:, off:512], ones_col[:],
                                        pt[:, base + off:base + 512],
                                        start=first, stop=last,
                                        skip_group_check=True)

                            for pr in range(nprs):
                                emit_qk(pr)
                                if pr == 0:
                                    flush_tail()
                                if pr >= 1:
                                    emit_pv(pr - 1)
                            emit_pv(nprs - 1)
                            pending[0] = (pv, dn, h, b, j)
                            if h == 0 and b == 0:
                                # drip batch-1 rope stage 2 into this
                                # PE-dense stretch
                                for _ in range(3):
                                    if stage2[1]:
                                        stage2[1].pop(0)()
                    while stage2[1]:
                        stage2[1].pop(0)()
                    flush_tail()
                    nc.gpsimd.collective_compute(
                        "AllToAll", ALU.bypass,
                        replica_groups=[list(range(N_CORES))],
                        ins=[a2a_in[h].opt()], outs=[a2a_out[h].opt()],
                        cc_dim="Partition")
                    nc.sync.dma_start(
                        oin[h][:].rearrange("p (s t) -> p s t", t=512),
                        a2a_out[h][:].rearrange("(s p) t -> p s t", p=128))

            # ====================== out projection ====================
            # pass A: h=0 partial sums for all (n, mp) -> SBUF (runs during
            # the second all-to-all); pass B: h=1 partials + DVE combine.
            ovs = {h: oin[h][:].rearrange("p (s t) -> p s t", t=512)
                   for h in range(2)}
            with tc.tile_pool(name="opool", bufs=1) as opool, \
                 tc.tile_pool(name="ops", bufs=2, space="PSUM") as ops:
                ph0 = {}
                for n in range(4):
                    for mp in range(4):
                        pso = ops.tile([128, 512], F32, tag=f"oa{mp}",
                                       name=f"oa{mp}")
                        for s in range(8):
                            nc.tensor.matmul(
                                pso[:],
                                ovs[0][:, s, 128 * mp:128 * (mp + 1)],
                                wov[:, 2 * s, 512 * n:512 * (n + 1)],
                                start=(s == 0), stop=(s == 7))
                        pt0 = opool.tile([128, 512], F32, tag="ph0",
                                         bufs=16, name="pt0")
                        nc.scalar.copy(pt0[:], pso[:])
                        ph0[(n, mp)] = pt0
                for n in range(4):
                    for mp in range(4):
                        pso = ops.tile([128, 512], F32, tag=f"oa{mp}",
                                       name=f"ob{mp}")
                        for s in range(8):
                            nc.tensor.matmul(
                                pso[:],
                                ovs[1][:, s, 128 * mp:128 * (mp + 1)],
                                wov[:, 2 * s + 1, 512 * n:512 * (n + 1)],
                                start=(s == 0), stop=(s == 7))
                        os_t = opool.tile([128, 512], F32, tag="osb",
                                          bufs=4, name="os_t")
                        nc.vector.tensor_add(os_t[:], pso[:],
                                             ph0[(n, mp)][:])
                        nc.sync.dma_start(
                            out.ap()[128 * mp:128 * (mp + 1),
                                     512 * n:512 * (n + 1)],
                            os_t[:])
            wpool_ctx.__exit__(None, None, None)
            bcall_ctx.__exit__(None, None, None)

    nc.compile()
    return nc


_NC_CACHE = None


def _get_nc():
    global _NC_CACHE
    if _NC_CACHE is None:
        _NC_CACHE = _build()
    return _NC_CACHE


def _host_prep(inputs):
    hs = np.asarray(inputs["hidden_states"], dtype=np.float32)
    Wq = np.asarray(inputs["Wq"], dtype=np.float32)
    Wk = np.asarray(inputs["Wk"], dtype=np.float32)
    Wv = np.asarray(inputs["Wv"], dtype=np.float32)
    Wo = np.asarray(inputs["Wo"], dtype=np.float32)
    cqw = np.asarray(inputs["canon_q_w"], dtype=np.float32)
    ckw = np.asarray(inputs["canon_k_w"], dtype=np.float32)
    cvw = np.asarray(inputs["canon_v_w"], dtype=np.float32)
    qnw = np.asarray(inputs["q_norm_w"], dtype=np.float32)
    knw = np.asarray(inputs["k_norm_w"], dtype=np.float32)

    bf = ml_dtypes.bfloat16
    hsT = np.ascontiguousarray(
        np.concatenate([hs[0].T, hs[1].T], axis=1)).astype(bf)
    WqT, WkT, WvT = Wq.T, Wk.T, Wv.T
    woT = np.ascontiguousarray(Wo.T).astype(bf)

    inv_freq = 1.0 / (10000.0 ** (np.arange(0, DH, 2, dtype=np.float64) / DH))
    freqs = np.arange(S, dtype=np.float64)[:, None] * inv_freq
    emb = np.concatenate([freqs, freqs], axis=-1)
    cosT, sinT = np.cos(emb).T, np.sin(emb).T

    def make_rope(normw, scale):
        A = cosT * normw[:, None] * scale
        wswap = normw[(np.arange(DH) + 64) % DH]
        sign = np.where(np.arange(DH) < 64, -1.0, 1.0)
        Bc = sinT * wswap[:, None] * sign[:, None] * scale
        return (np.ascontiguousarray(A).astype(bf),
                np.ascontiguousarray(Bc).astype(bf))

    Aq, Bq = make_rope(qnw, SCALE)
    Ak, Bk = make_rope(knw, 1.0)

    p = np.arange(128)[:, None]
    f = np.arange(128)[None, :]
    maskd = np.where(p <= f, 0.0, NEG).astype(np.float32)
    maskTb = np.ascontiguousarray(maskd.T).astype(bf)
    idb = np.eye(128, dtype=np.float32).astype(bf)

    in_maps = []
    for r in range(N_CORES):
        wTc = np.ascontiguousarray(np.concatenate(
            [WqT[:, 256 * r:256 * r + 256],
             WkT[:, 128 * r:128 * r + 128],
             WvT[:, 128 * r:128 * r + 128]], axis=1)).astype(bf)
        cwc = np.ascontiguousarray(np.concatenate(
            [cqw[256 * r:256 * r + 256],
             ckw[128 * r:128 * r + 128],
             cvw[128 * r:128 * r + 128]], axis=0)).astype(np.float32)
        in_maps.append({
            "hsT": hsT, "wT": wTc, "woT": woT, "cw": cwc,
            "ropeAq": Aq, "ropeBq": Bq, "ropeAk": Ak, "ropeBk": Bk,
            "maskTb": maskTb, "idb": idb,
        })
    return in_maps


def kernel(**inputs):
    nc = _get_nc()
    in_maps = _host_prep(inputs)
    res = run_bass_kernel_spmd(nc, in_maps, core_ids=list(range(N_CORES)))
    full = np.empty((B, S, D), np.float32)
    for r in range(N_CORES):
        full[r // 4, 512 * (r % 4):512 * (r % 4 + 1), :] = res.results[r]["out"]
    return full


# revision 24
# speedup vs baseline: 1.4724x; 1.0256x over previous
"""Trainium2 Bass kernel for CanonCausalMultiheadAttn.

Sharding: tensor-parallel over heads across 8 cores (2 q-heads + 1 kv-head
per core), both batches replicated. Two head-split AllToAlls exchange
attention outputs so each core owns one (batch, seq-slice) of the final
output projection; the first overlaps the second half of attention and the
second overlaps the h=0 partial pass of the output projection.

Per-core pipeline (shapes hardcoded for B=2, S=2048, D=2048):
  QKV proj (bf16 matmul, weights SBUF-resident) -> canon conv via halo'd
  raw buffer (DVE, bf16) -> qk rmsnorm rstd via PE column-sum matmuls and
  fast-approx reciprocal -> RoPE (DVE bf16; norm-weight & 1/sqrt(dh)
  folded into host tables; q AND k rstd folded in via K=1 broadcast
  matmuls whose emission is software-pipelined into later PE-dense
  phases) -> causal attention with scores in [Sk, Sq] layout, two
  Sk-blocks paired per [128,1024] PSUM tile so one exp covers both; the
  causal mask and the fully-masked wedge are added on the PE itself
  (maskT.T @ I accumulation) so exp depends only on the PE; PV computed
  transposed (V stationary) directly in [dh, Sq]; softmax denominator via
  ones-column matmuls; normalize tail software-pipelined into the next
  block -> AllToAll x2 (head-split) -> output projection in two passes
  (h=0 partials to SBUF during the second AllToAll, then h=1 + combine).
"""
import sys

sys.path.insert(0, '/opt/trn_rl_repo')

import numpy as np
import ml_dtypes

import concourse.bass as bass
import concourse.mybir as mybir
import concourse.tile as tile
from concourse import bacc
from concourse.bass_utils import run_bass_kernel_spmd

F32 = mybir.dt.float32
F32R = mybir.dt.float32r
BF16 = mybir.dt.bfloat16
AF = mybir.ActivationFunctionType
ALU = mybir.AluOpType

B, S, D = 2, 2048, 2048
NH, NKV, DH = 16, 8, 128
K_CONV = 4
EPS = 1e-6
SCALE = 1.0 / float(np.sqrt(DH))
NEG = -1e9
N_CORES = 8
NCB = S // 512          # 512-token chunks per batch
N_SKB = S // 128        # Sk blocks per batch


def _build():
    nc = bacc.Bacc("TRN2", target_bir_lowering=False, debug=False,
                   num_devices=N_CORES)

    hsT = nc.dram_tensor("hsT", [D, B * S], BF16, kind="ExternalInput")
    wT = nc.dram_tensor("wT", [D, 512], BF16, kind="ExternalInput")
    woT = nc.dram_tensor("woT", [D, D], BF16, kind="ExternalInput")
    cw = nc.dram_tensor("cw", [512, K_CONV], F32, kind="ExternalInput")
    ropeAq = nc.dram_tensor("ropeAq", [DH, S], BF16, kind="ExternalInput")
    ropeBq = nc.dram_tensor("ropeBq", [DH, S], BF16, kind="ExternalInput")
    ropeAk = nc.dram_tensor("ropeAk", [DH, S], BF16, kind="ExternalInput")
    ropeBk = nc.dram_tensor("ropeBk", [DH, S], BF16, kind="ExternalInput")
    maskTb = nc.dram_tensor("maskTb", [128, 128], BF16, kind="ExternalInput")
    idb = nc.dram_tensor("idb", [128, 128], BF16, kind="ExternalInput")
    out = nc.dram_tensor("out", [512, D], F32, kind="ExternalOutput")

    with tile.TileContext(nc) as tc:
        with tc.tile_pool(name="const", bufs=1) as cpool, \
             tc.tile_pool(name="persist", bufs=1) as pers, \
             tc.tile_pool(name="dram", bufs=1, space="DRAM") as dram:

            # QKV weights resident in SBUF: [128, 16 k-blocks x 512]
            wT_sb = cpool.tile([128, 16 * 512], BF16, tag="wTsb")
            wv = wT_sb[:].rearrange("p (k c) -> p k c", c=512)
            for kk in range(4):
                nc.sync.dma_start(
                    wv[:, 4 * kk:4 * (kk + 1), :],
                    wT.ap()[512 * kk:512 * (kk + 1), :]
                    .rearrange("(k p) c -> p k c", p=128))

            # ---- constants (rope tables last: not needed until late) ----
            mask_sb = cpool.tile([128, 128], BF16, tag="mask")
            nc.sync.dma_start(mask_sb[:], maskTb.ap())
            id_sb = cpool.tile([128, 128], BF16, tag="idsb")
            nc.sync.dma_start(id_sb[:], idb.ap())
            cw_sb = []
            for mt in range(4):
                t = cpool.tile([128, K_CONV], F32, tag=f"cw{mt}",
                               name=f"cw{mt}")
                nc.sync.dma_start(t[:], cw.ap()[128 * mt:128 * mt + 128, :])
                cw_sb.append(t)
            ones_col = cpool.tile([128, 1], BF16, tag="oc")
            nc.vector.memset(ones_col[:], 1.0)
            eps_sb = cpool.tile([1, 1], F32, tag="eps")
            nc.vector.memset(eps_sb[:], EPS)
            ones_row = cpool.tile([1, 128], F32, tag="or")
            nc.vector.memset(ones_row[:], 1.0)
            # NEG broadcast: negrow.T @ onesb[:, :w] == NEG everywhere
            negrow = cpool.tile([128, 128], BF16, tag="negrow")
            nc.vector.memset(negrow[:], 0.0)
            nc.vector.memset(negrow[0:1, :], NEG)
            onesb = cpool.tile([128, 512], BF16, tag="onesb")
            nc.vector.memset(onesb[:], 1.0)
            s0_sb = []
            for mt in range(4):
                t = cpool.tile([128, 1], F32, tag=f"s0{mt}", name=f"s0{mt}")
                nc.vector.tensor_scalar_add(t[:], cw_sb[mt][:, 0:1], 1.0)
                s0_sb.append(t)
            ropes = {}
            for nm in ("Aq", "Bq", "Ak", "Bk"):
                ropes[nm] = cpool.tile([DH, S], BF16, tag=f"rope{nm}",
                                       name=f"rope{nm}")

            # persistent per-(b,mt) tiles
            roped = {}   # (b, mt) -> [128, S] bf16 (rstd folded in)
            vT = {}      # b -> [128, N_SKB*128] bf16 (V transposed blocks)

            for b in range(B):
                vT[b] = pers.tile([128, N_SKB * 128], BF16, tag=f"vT{b}",
                                  name=f"vT{b}")
                for mt in range(3):
                    roped[(b, mt)] = pers.tile(
                        [128, S], BF16, tag=f"roped{b}{mt}",
                        name=f"roped{b}{mt}")

            # ============ phase Q: QKV + canon + norm + rope ============
            with tc.tile_pool(name="qps", bufs=1, space="PSUM") as qps, \
                 tc.tile_pool(name="spp", bufs=2, space="PSUM") as spp, \
                 tc.tile_pool(name="bps", bufs=2, space="PSUM") as bps, \
                 tc.tile_pool(name="bwork", bufs=1) as bw:
                rn_sb = {}   # mt -> [1, S] f32 rstd rows
                for mt in range(3):
                    rn_sb[mt] = bw.tile(
                        [1, S], F32, tag=f"rn{mt}", name=f"rn{mt}")
                cn = {}
                raw_h = {}
                for mt in range(4):
                    cn[mt] = bw.tile([128, S], BF16, tag=f"cn{mt}",
                                     name=f"cn{mt}")
                    raw_h[mt] = bw.tile([128, 516], BF16,
                                        tag=f"rawh{mt}",
                                        name=f"raw_h{mt}")
                psums = [qps.tile([128, 512], F32, tag=f"qk{mt}",
                                  name=f"qk{mt}") for mt in range(4)]
                for b in range(B):
                    if True:
                        for mt in range(4):
                            nc.vector.memset(raw_h[mt][:, 0:4], 0.0)

                        def emit_chunk_mms(n):
                            hs_sb = bw.tile([128, 16 * 512], BF16,
                                            tag="hschunk", bufs=2,
                                            name="hs_sb")
                            nc.sync.dma_start(
                                hs_sb[:].rearrange("p (k s) -> p k s", s=512),
                                hsT.ap()[:, b * S + 512 * n:
                                         b * S + 512 * (n + 1)]
                                .rearrange("(k p) s -> p k s", p=128))
                            hvv = hs_sb[:].rearrange("p (k s) -> p k s",
                                                     s=512)
                            for k in range(16):
                                for mt in range(4):
                                    nc.tensor.matmul(
                                        psums[mt][:],
                                        wv[:, k, 128 * mt:128 * (mt + 1)],
                                        hvv[:, k, :],
                                        start=(k == 0), stop=(k == 15))
                            if b == 0 and n == 1:
                                for nm, t in (("Aq", ropeAq), ("Bq", ropeBq),
                                              ("Ak", ropeAk), ("Bk", ropeBk)):
                                    nc.sync.dma_start(ropes[nm][:], t.ap())

                        def emit_canon(n):
                            lo = 512 * n
                            for mt in range(4):
                                rh = raw_h[mt]
                                if n > 0:
                                    nc.vector.tensor_copy(
                                        rh[:, 1:4], rh[:, 513:516])
                                nc.scalar.copy(rh[:, 4:516], psums[mt][:])
                                c = cn[mt]
                                nc.vector.tensor_scalar_mul(
                                    c[:, lo:lo + 512], rh[:, 4:516],
                                    s0_sb[mt][:])
                                for k in range(1, K_CONV):
                                    nc.vector.scalar_tensor_tensor(
                                        c[:, lo:lo + 512],
                                        rh[:, 4 - k:516 - k],
                                        cw_sb[mt][:, k:k + 1],
                                        c[:, lo:lo + 512],
                                        ALU.mult, ALU.add)
                            # squares for rmsnorm (q0, q1, k)
                            for mt in range(3):
                                sq = bw.tile([128, 512], BF16, tag="sqr",
                                             bufs=3, name="sq")
                                nc.vector.tensor_mul(
                                    sq[:], cn[mt][:, lo:lo + 512],
                                    cn[mt][:, lo:lo + 512])
                                sqs[(n, mt)] = sq

                        def emit_norm(n):
                            for mt in range(3):
                                sp = spp.tile([1, 512], F32, tag="ssq")
                                nc.tensor.matmul(sp[:], ones_col[:],
                                                 sqs.pop((n, mt))[:],
                                                 start=True, stop=True)
                                srt = bw.tile([1, 512], F32, tag="srt",
                                              bufs=2, name="srt")
                                nc.scalar.activation(srt[:], sp[:], AF.Sqrt,
                                                     bias=eps_sb[:],
                                                     scale=1.0 / DH)
                                nc.vector.reciprocal_approx_fast(
                                    rn_sb[mt][:, 512 * n:512 * (n + 1)],
                                    srt[:])
                            # V transpose blocks for this chunk
                            for t in range(4):
                                i = 4 * n + t
                                nc.sync.dma_start_transpose(
                                    vT[b][:, 128 * i:128 * (i + 1)],
                                    cn[3][:, 128 * i:128 * (i + 1)])

                        def emit_bc(n):
                            # rstd broadcast for chunk n (PE + ACT only;
                            # emitted inside the PE-dense chunk stream)
                            for mt in range(3):
                                bp = bps.tile([128, 512], F32, tag="bcp")
                                nc.tensor.matmul(
                                    bp[:], ones_row[:],
                                    rn_sb[mt][:, 512 * n:512 * (n + 1)],
                                    start=True, stop=True)
                                bcb = bw.tile([128, 512], BF16, tag="bcs",
                                              bufs=4, name="bcs")
                                nc.scalar.copy(bcb[:], bp[:])
                                bcb_store[(mt, n)] = bcb

                        def emit_rope1(n):
                            # chunk-wise rope + in-place rstd scale so
                            # roped is complete when the pipeline drains
                            lo = 512 * n
                            for mt in (2, 0, 1):
                                is_q = mt < 2
                                x = cn[mt]
                                A_ = ropes["Aq"] if is_q else ropes["Ak"]
                                B_ = ropes["Bq"] if is_q else ropes["Bk"]
                                sh = bw.tile([128, 512], BF16, tag="shift",
                                             bufs=3, name="sh")
                                nc.sync.dma_start(sh[0:64, :],
                                                  x[64:128, lo:lo + 512])
                                nc.sync.dma_start(sh[64:128, :],
                                                  x[0:64, lo:lo + 512])
                                nc.vector.tensor_mul(sh[:], sh[:],
                                                     B_[:, lo:lo + 512])
                                ro = roped[(b, mt)]
                                nc.vector.tensor_mul(ro[:, lo:lo + 512],
                                                     x[:, lo:lo + 512],
                                                     A_[:, lo:lo + 512])
                                nc.vector.tensor_add(ro[:, lo:lo + 512],
                                                     ro[:, lo:lo + 512],
                                                     sh[:])
                                nc.vector.tensor_mul(
                                    ro[:, lo:lo + 512],
                                    ro[:, lo:lo + 512],
                                    bcb_store.pop((mt, n))[:])

                        sqs = {}
                        bcb_store = {}
                        for n in range(NCB + 2):
                            if n < NCB:
                                emit_chunk_mms(n)
                            if 1 <= n <= NCB:
                                emit_norm(n - 1)
                            if n >= 2:
                                emit_bc(n - 2)
                            if n < NCB:
                                emit_canon(n)
                            if n >= 2:
                                emit_rope1(n - 2)

            # ============ attention + head-split all-to-all ============
            wpool_ctx = tc.tile_pool(name="wpool", bufs=1)
            wpool = wpool_ctx.__enter__()
            # Wo resident prefetch (needed only for the output projection)
            wo_sb = wpool.tile([128, 16 * D], BF16, tag="wosb")
            wov = wo_sb[:].rearrange("p (g o) -> p g o", o=D)
            for gg in range(4):
                nc.sync.dma_start(
                    wov[:, 4 * gg:4 * (gg + 1), :],
                    woT.ap()[512 * gg:512 * (gg + 1), :]
                    .rearrange("(g p) o -> p g o", p=128))

            a2a_in = {}
            a2a_out = {}
            oin = {}
            for h in range(2):
                a2a_in[h] = dram.tile([1024, 512], BF16, tag=f"a2ain{h}",
                                      name=f"a2a_in{h}")
                a2a_out[h] = dram.tile([1024, 512], BF16, tag=f"a2aout{h}",
                                       name=f"a2a_out{h}")
                oin[h] = wpool.tile([128, 8 * 512], BF16, tag=f"oin{h}",
                                    name=f"oin{h}")

            with tc.tile_pool(name="scps", bufs=2, space="PSUM") as scps, \
                 tc.tile_pool(name="pvps", bufs=2, space="PSUM") as pvps, \
                 tc.tile_pool(name="dnps", bufs=1, space="PSUM") as dnps, \
                 tc.tile_pool(name="bcps", bufs=1, space="PSUM") as bcps, \
                 tc.tile_pool(name="apool", bufs=1) as apool:
                pending = [None]

                def flush_tail():
                    if pending[0] is None:
                        return
                    pv, dn, h, b, j = pending[0]
                    pending[0] = None
                    rec = apool.tile([1, 512], F32, tag="rec",
                                     bufs=2, name="rec")
                    nc.vector.reciprocal_approx_fast(rec[:], dn[:])
                    bc = bcps.tile([128, 512], F32, tag="bc", name="bc")
                    nc.tensor.matmul(bc[:], ones_row[:], rec[:],
                                     start=True, stop=True)
                    bcb = apool.tile([128, 512], BF16, tag="bcbn",
                                     bufs=2, name="bcb")
                    nc.vector.tensor_copy(bcb[:], bc[:])
                    nrm = apool.tile([128, 512], BF16, tag="nrm",
                                     bufs=2, name="nrm")
                    nc.vector.tensor_mul(nrm[:], pv[:], bcb[:])
                    nc.sync.dma_start(
                        a2a_in[h][128 * (4 * b + j):
                                  128 * (4 * b + j + 1), :],
                        nrm[:])

                for h in range(2):
                    for b in range(B):
                        KT = roped[(b, 2)]
                        QT = roped[(b, h)]
                        vt = vT[b]
                        for j in range(NCB):
                            pv = pvps.tile([128, 512], F32, tag="pv",
                                           name="pv")
                            dn = dnps.tile([1, 512], F32, tag="dn",
                                           name="dn")
                            acc = apool.tile([128, 512], BF16, tag="acc",
                                             bufs=2, name="acc")
                            nprs = 2 * j + 2   # pairs of Sk blocks
                            pts = [None] * nprs
                            offp = [None] * nprs

                            def emit_qk(pr):
                                ps = scps.tile([128, 1024], F32, tag="sc",
                                               name="ps")
                                pt = apool.tile([128, 1024], BF16, tag="p",
                                                bufs=4, name="pt")
                                offs = []
                                for half in range(2):
                                    i = 2 * pr + half
                                    r = i - 4 * j
                                    off = 128 * max(r, 0)
                                    offs.append(off)
                                    base = 512 * half
                                    diag = (r >= 0)
                                    nc.tensor.matmul(
                                        ps[:, base + off:base + 512],
                                        KT[:, 128 * i:128 * (i + 1)],
                                        QT[:, 512 * j + off:512 * (j + 1)],
                                        start=True, stop=not diag)
                                    if diag:
                                        nc.tensor.matmul(
                                            ps[:, base + off:
                                               base + off + 128],
                                            mask_sb[:], id_sb[:],
                                            start=False, stop=True,
                                            skip_group_check=True)
                                if offs[1] > 0:
                                    nc.tensor.matmul(
                                        ps[:, 512:512 + offs[1]],
                                        negrow[:], onesb[:, 0:offs[1]],
                                        start=True, stop=True)
                                nc.scalar.activation(
                                    pt[:, offs[0]:1024],
                                    ps[:, offs[0]:1024], AF.Exp)
                                pts[pr] = pt
                                offp[pr] = offs
                                # denominator partials on DVE
                                if pr == 0:
                                    nc.vector.tensor_copy(
                                        acc[:], pt[:, 0:512])
                                else:
                                    nc.vector.tensor_add(
                                        acc[:, offs[0]:512],
                                        acc[:, offs[0]:512],
                                        pt[:, offs[0]:512])
                                nc.vector.tensor_add(
                                    acc[:, offs[1]:512],
                                    acc[:, offs[1]:512],
                                    pt[:, 512 + offs[1]:1024])

                            def emit_pv(pr):
                                pt = pts[pr]
                                offs = offp[pr]
                                for half in range(2):
                                    i = 2 * pr + half
                                    off = offs[half]
                                    first = (i == 0)
                                    last = (i == 4 * j + 3)
                                    base = 512 * half
                                    nc.tensor.matmul(
                                        pv[:, off:512],
                                        vt[:, 128 * i:128 * (i + 1)],
                                        pt[:, base + off:base + 512],
                                        start=first, stop=last,
                                        skip_group_check=True)

                            for pr in range(nprs):
                                emit_qk(pr)
                                if pr == 0:
                                    flush_tail()
                                if pr >= 1:
                                    emit_pv(pr - 1)
                            emit_pv(nprs - 1)
                            nc.tensor.matmul(dn[:], ones_col[:], acc[:],
                                             start=True, stop=True)
                            pending[0] = (pv, dn, h, b, j)
                    flush_tail()
                    nc.gpsimd.collective_compute(
                        "AllToAll", ALU.bypass,
                        replica_groups=[list(range(N_CORES))],
                        ins=[a2a_in[h].opt()], outs=[a2a_out[h].opt()],
                        cc_dim="Partition")
                    nc.sync.dma_start(
                        oin[h][:].rearrange("p (s t) -> p s t", t=512),
                        a2a_out[h][:].rearrange("(s p) t -> p s t", p=128))

            # ====================== out projection ====================
            # pass A: h=0 partial sums for all (n, mp) -> SBUF (runs during
            # the second all-to-all); pass B: h=1 partials + DVE combine.
            ovs = {h: oin[h][:].rearrange("p (s t) -> p s t", t=512)
                   for h in range(2)}
            with tc.tile_pool(name="opool", bufs=1) as opool, \
                 tc.tile_pool(name="ops", bufs=2, space="PSUM") as ops:
                ph0 = {}
                for n in range(4):
                    for mp in range(4):
                        pso = ops.tile([128, 512], F32, tag=f"oa{mp}",
                                       name=f"oa{mp}")
                        for s in range(8):
                            nc.tensor.matmul(
                                pso[:],
                                ovs[0][:, s, 128 * mp:128 * (mp + 1)],
                                wov[:, 2 * s, 512 * n:512 * (n + 1)],
                                start=(s == 0), stop=(s == 7))
                        pt0 = opool.tile([128, 512], F32, tag="ph0",
                                         bufs=16, name="pt0")
                        nc.scalar.copy(pt0[:], pso[:])
                        ph0[(n, mp)] = pt0
                for n in range(4):
                    for mp in range(4):
                        pso = ops.tile([128, 512], F32, tag=f"oa{mp}",
                                       name=f"ob{mp}")
                        for s in range(8):
                            nc.tensor.matmul(
                                pso[:],
                                ovs[1][:, s, 128 * mp:128 * (mp + 1)],
                                wov[:, 2 * s + 1, 512 * n:512 * (n + 1)],
                                start=(s == 0), stop=(s == 7))
                        os_t = opool.tile([128, 512], F32, tag="osb",
                                          bufs=4, name="os_t")
                        nc.vector.tensor_add(os_t[:], pso[:],
                                             ph0[(n, mp)][:])
                        nc.sync.dma_start(
                            out.ap()[128 * mp:128 * (mp + 1),
                                     512 * n:512 * (n + 1)],
                            os_t[:])
            wpool_ctx.__exit__(None, None, None)

    nc.compile()
    return nc


_NC_CACHE = None


def _get_nc():
    global _NC_CACHE
    if _NC_CACHE is None:
        _NC_CACHE = _build()
    return _NC_CACHE


def _host_prep(inputs):
    hs = np.asarray(inputs["hidden_states"], dtype=np.float32)
    Wq = np.asarray(inputs["Wq"], dtype=np.float32)
    Wk = np.asarray(inputs["Wk"], dtype=np.float32)
    Wv = np.asarray(inputs["Wv"], dtype=np.float32)
    Wo = np.asarray(inputs["Wo"], dtype=np.float32)
    cqw = np.asarray(inputs["canon_q_w"], dtype=np.float32)
    ckw = np.asarray(inputs["canon_k_w"], dtype=np.float32)
    cvw = np.asarray(inputs["canon_v_w"], dtype=np.float32)
    qnw = np.asarray(inputs["q_norm_w"], dtype=np.float32)
    knw = np.asarray(inputs["k_norm_w"], dtype=np.float32)

    bf = ml_dtypes.bfloat16
    hsT = np.ascontiguousarray(
        np.concatenate([hs[0].T, hs[1].T], axis=1)).astype(bf)
    WqT, WkT, WvT = Wq.T, Wk.T, Wv.T
    woT = np.ascontiguousarray(Wo.T).astype(bf)

    inv_freq = 1.0 / (10000.0 ** (np.arange(0, DH, 2, dtype=np.float64) / DH))
    freqs = np.arange(S, dtype=np.float64)[:, None] * inv_freq
    emb = np.concatenate([freqs, freqs], axis=-1)
    cosT, sinT = np.cos(emb).T, np.sin(emb).T

    def make_rope(normw, scale):
        A = cosT * normw[:, None] * scale
        wswap = normw[(np.arange(DH) + 64) % DH]
        sign = np.where(np.arange(DH) < 64, -1.0, 1.0)
        Bc = sinT * wswap[:, None] * sign[:, None] * scale
        return (np.ascontiguousarray(A).astype(bf),
                np.ascontiguousarray(Bc).astype(bf))

    Aq, Bq = make_rope(qnw, SCALE)
    Ak, Bk = make_rope(knw, 1.0)

    p = np.arange(128)[:, None]
    f = np.arange(128)[None, :]
    maskd = np.where(p <= f, 0.0, NEG).astype(np.float32)
    maskTb = np.ascontiguousarray(maskd.T).astype(bf)
    idb = np.eye(128, dtype=np.float32).astype(bf)

    in_maps = []
    for r in range(N_CORES):
        wTc = np.ascontiguousarray(np.concatenate(
            [WqT[:, 256 * r:256 * r + 256],
             WkT[:, 128 * r:128 * r + 128],
             WvT[:, 128 * r:128 * r + 128]], axis=1)).astype(bf)
        cwc = np.ascontiguousarray(np.concatenate(
            [cqw[256 * r:256 * r + 256],
             ckw[128 * r:128 * r + 128],
             cvw[128 * r:128 * r + 128]], axis=0)).astype(np.float32)
        in_maps.append({
            "hsT": hsT, "wT": wTc, "woT": woT, "cw": cwc,
            "ropeAq": Aq, "ropeBq": Bq, "ropeAk": Ak, "ropeBk": Bk,
            "maskTb": maskTb, "idb": idb,
        })
    return in_maps


def kernel(**inputs):
    nc = _get_nc()
    in_maps = _host_prep(inputs)
    res = run_bass_kernel_spmd(nc, in_maps, core_ids=list(range(N_CORES)))
    full = np.empty((B, S, D), np.float32)
    for r in range(N_CORES):
        full[r // 4, 512 * (r % 4):512 * (r % 4 + 1), :] = res.results[r]["out"]
    return full


# revision 29
# speedup vs baseline: 1.6033x; 1.0888x over previous
"""Trainium2 Bass kernel for CanonCausalMultiheadAttn.

Sharding: tensor-parallel over heads across 8 cores (2 q-heads + 1 kv-head
per core), both batches replicated. Two head-split AllToAlls exchange
attention outputs so each core owns one (batch, seq-slice) of the final
output projection; the first overlaps the second half of attention and the
second overlaps the h=0 partial pass of the output projection.

Per-core pipeline (shapes hardcoded for B=2, S=2048, D=2048):
  QKV proj (bf16 matmul, weights SBUF-resident) -> canon conv via halo'd
  raw buffer (DVE, bf16) -> qk rmsnorm rstd via PE column-sum matmuls and
  fast-approx reciprocal -> RoPE (DVE bf16; norm-weight & 1/sqrt(dh)
  folded into host tables; q AND k rstd folded in via K=1 broadcast
  matmuls whose emission is software-pipelined into later PE-dense
  phases) -> causal attention with scores in [Sk, Sq] layout, two
  Sk-blocks paired per [128,1024] PSUM tile so one exp covers both; the
  causal mask and the fully-masked wedge are added on the PE itself
  (maskT.T @ I accumulation) so exp depends only on the PE; PV computed
  transposed (V stationary) directly in [dh, Sq]; softmax denominator via
  ones-column matmuls; normalize tail software-pipelined into the next
  block -> AllToAll x2 (head-split) -> output projection in two passes
  (h=0 partials to SBUF during the second AllToAll, then h=1 + combine).
"""
import sys

sys.path.insert(0, '/opt/trn_rl_repo')

import numpy as np
import ml_dtypes

import concourse.bass as bass
import concourse.mybir as mybir
import concourse.tile as tile
from concourse import bacc
from concourse.bass_utils import run_bass_kernel_spmd

F32 = mybir.dt.float32
F32R = mybir.dt.float32r
BF16 = mybir.dt.bfloat16
AF = mybir.ActivationFunctionType
ALU = mybir.AluOpType

B, S, D = 2, 2048, 2048
NH, NKV, DH = 16, 8, 128
K_CONV = 4
EPS = 1e-6
SCALE = 1.0 / float(np.sqrt(DH))
NEG = -1e9
N_CORES = 8
NCB = S // 512          # 512-token chunks per batch
N_SKB = S // 128        # Sk blocks per batch


def _build():
    nc = bacc.Bacc("TRN2", target_bir_lowering=False, debug=False,
                   num_devices=N_CORES)

    hsT = nc.dram_tensor("hsT", [D, B * S], BF16, kind="ExternalInput")
    wT = nc.dram_tensor("wT", [D, 512], BF16, kind="ExternalInput")
    woT = nc.dram_tensor("woT", [D, D], BF16, kind="ExternalInput")
    cw = nc.dram_tensor("cw", [512, K_CONV], F32, kind="ExternalInput")
    ropeAq = nc.dram_tensor("ropeAq", [DH, S], BF16, kind="ExternalInput")
    ropeBq = nc.dram_tensor("ropeBq", [DH, S], BF16, kind="ExternalInput")
    ropeAk = nc.dram_tensor("ropeAk", [DH, S], BF16, kind="ExternalInput")
    ropeBk = nc.dram_tensor("ropeBk", [DH, S], BF16, kind="ExternalInput")
    maskTb = nc.dram_tensor("maskTb", [128, 128], BF16, kind="ExternalInput")
    idb = nc.dram_tensor("idb", [128, 128], BF16, kind="ExternalInput")
    out = nc.dram_tensor("out", [512, D], F32, kind="ExternalOutput")

    with tile.TileContext(nc) as tc:
        with tc.tile_pool(name="const", bufs=1) as cpool, \
             tc.tile_pool(name="persist", bufs=1) as pers, \
             tc.tile_pool(name="dram", bufs=1, space="DRAM") as dram:

            # QKV weights resident in SBUF: [128, 16 k-blocks x 512]
            wT_sb = cpool.tile([128, 16 * 512], BF16, tag="wTsb")
            wv = wT_sb[:].rearrange("p (k c) -> p k c", c=512)
            for kk in range(4):
                nc.sync.dma_start(
                    wv[:, 4 * kk:4 * (kk + 1), :],
                    wT.ap()[512 * kk:512 * (kk + 1), :]
                    .rearrange("(k p) c -> p k c", p=128))

            # ---- constants (rope tables last: not needed until late) ----
            mask_sb = cpool.tile([128, 128], BF16, tag="mask")
            nc.sync.dma_start(mask_sb[:], maskTb.ap())
            id_sb = cpool.tile([128, 128], BF16, tag="idsb")
            nc.sync.dma_start(id_sb[:], idb.ap())
            cw_sb = []
            for mt in range(4):
                t = cpool.tile([128, K_CONV], F32, tag=f"cw{mt}",
                               name=f"cw{mt}")
                nc.sync.dma_start(t[:], cw.ap()[128 * mt:128 * mt + 128, :])
                cw_sb.append(t)
            ones_col = cpool.tile([128, 1], BF16, tag="oc")
            nc.vector.memset(ones_col[:], 1.0)
            eps_sb = cpool.tile([1, 1], F32, tag="eps")
            nc.vector.memset(eps_sb[:], EPS)
            ones_row = cpool.tile([1, 128], F32, tag="or")
            nc.vector.memset(ones_row[:], 1.0)
            # NEG broadcast: negrow.T @ onesb[:, :w] == NEG everywhere
            negrow = cpool.tile([128, 128], BF16, tag="negrow")
            nc.vector.memset(negrow[:], 0.0)
            nc.vector.memset(negrow[0:1, :], NEG)
            onesb = cpool.tile([128, 512], BF16, tag="onesb")
            nc.vector.memset(onesb[:], 1.0)
            s0_sb = []
            for mt in range(4):
                t = cpool.tile([128, 1], F32, tag=f"s0{mt}", name=f"s0{mt}")
                nc.vector.tensor_scalar_add(t[:], cw_sb[mt][:, 0:1], 1.0)
                s0_sb.append(t)
            ropes = {}
            for nm in ("Aq", "Bq", "Ak", "Bk"):
                ropes[nm] = cpool.tile([DH, S], BF16, tag=f"rope{nm}",
                                       name=f"rope{nm}")

            # persistent per-(b,mt) tiles
            roped = {}   # (b, mt) -> [128, S] bf16 (rstd folded in)
            vT = {}      # b -> [128, N_SKB*128] bf16 (V transposed blocks)

            for b in range(B):
                vT[b] = pers.tile([128, N_SKB * 128], BF16, tag=f"vT{b}",
                                  name=f"vT{b}")
                for mt in range(3):
                    roped[(b, mt)] = pers.tile(
                        [128, S], BF16, tag=f"roped{b}{mt}",
                        name=f"roped{b}{mt}")

            # ============ phase Q: QKV + canon + norm + rope ============
            with tc.tile_pool(name="qps", bufs=1, space="PSUM") as qps, \
                 tc.tile_pool(name="spp", bufs=2, space="PSUM") as spp, \
                 tc.tile_pool(name="bps", bufs=2, space="PSUM") as bps, \
                 tc.tile_pool(name="bwork", bufs=1) as bw:
                rn_sb = {}   # mt -> [1, S] f32 rstd rows
                for mt in range(3):
                    rn_sb[mt] = bw.tile(
                        [1, S], F32, tag=f"rn{mt}", name=f"rn{mt}")
                cn = {}
                raw_h = {}
                for mt in range(4):
                    cn[mt] = bw.tile([128, S], BF16, tag=f"cn{mt}",
                                     name=f"cn{mt}")
                    raw_h[mt] = bw.tile([128, 516], BF16,
                                        tag=f"rawh{mt}",
                                        name=f"raw_h{mt}")
                psums = [qps.tile([128, 512], F32, tag=f"qk{mt}",
                                  name=f"qk{mt}") for mt in range(4)]
                for b in range(B):
                    if True:
                        for mt in range(4):
                            nc.vector.memset(raw_h[mt][:, 0:4], 0.0)

                        def emit_chunk_mms(n):
                            hs_sb = bw.tile([128, 16 * 512], BF16,
                                            tag="hschunk", bufs=2,
                                            name="hs_sb")
                            nc.sync.dma_start(
                                hs_sb[:].rearrange("p (k s) -> p k s", s=512),
                                hsT.ap()[:, b * S + 512 * n:
                                         b * S + 512 * (n + 1)]
                                .rearrange("(k p) s -> p k s", p=128))
                            hvv = hs_sb[:].rearrange("p (k s) -> p k s",
                                                     s=512)
                            for k in range(16):
                                for mt in range(4):
                                    nc.tensor.matmul(
                                        psums[mt][:],
                                        wv[:, k, 128 * mt:128 * (mt + 1)],
                                        hvv[:, k, :],
                                        start=(k == 0), stop=(k == 15))
                            if b == 0 and n == 1:
                                for nm, t in (("Aq", ropeAq), ("Bq", ropeBq),
                                              ("Ak", ropeAk), ("Bk", ropeBk)):
                                    nc.sync.dma_start(ropes[nm][:], t.ap())

                        def emit_canon(n):
                            lo = 512 * n
                            for mt in range(4):
                                rh = raw_h[mt]
                                if n > 0:
                                    nc.vector.tensor_copy(
                                        rh[:, 1:4], rh[:, 513:516])
                                nc.vector.tensor_copy(rh[:, 4:516],
                                                      psums[mt][:])
                                c = cn[mt]
                                nc.vector.tensor_scalar_mul(
                                    c[:, lo:lo + 512], rh[:, 4:516],
                                    s0_sb[mt][:])
                                for k in range(1, K_CONV):
                                    nc.vector.scalar_tensor_tensor(
                                        c[:, lo:lo + 512],
                                        rh[:, 4 - k:516 - k],
                                        cw_sb[mt][:, k:k + 1],
                                        c[:, lo:lo + 512],
                                        ALU.mult, ALU.add)
                            # squares for rmsnorm (q0, q1, k)
                            for mt in range(3):
                                sq = bw.tile([128, 512], BF16, tag="sqr",
                                             bufs=3, name="sq")
                                nc.vector.tensor_mul(
                                    sq[:], cn[mt][:, lo:lo + 512],
                                    cn[mt][:, lo:lo + 512])
                                sqs[(n, mt)] = sq

                        def emit_norm(n):
                            for mt in range(3):
                                sp = spp.tile([1, 512], F32, tag="ssq")
                                nc.tensor.matmul(sp[:], ones_col[:],
                                                 sqs.pop((n, mt))[:],
                                                 start=True, stop=True)
                                srt = bw.tile([1, 512], F32, tag="srt",
                                              bufs=2, name="srt")
                                nc.scalar.activation(srt[:], sp[:], AF.Sqrt,
                                                     bias=eps_sb[:],
                                                     scale=1.0 / DH)
                                nc.vector.reciprocal_approx_fast(
                                    rn_sb[mt][:, 512 * n:512 * (n + 1)],
                                    srt[:])
                            # V transpose blocks for this chunk (scalar
                            # queue: keeps the sync queue free for hs)
                            for t in range(4):
                                i = 4 * n + t
                                nc.scalar.dma_start_transpose(
                                    vT[b][:, 128 * i:128 * (i + 1)],
                                    cn[3][:, 128 * i:128 * (i + 1)])

                        def emit_bc(n):
                            # rstd broadcast for chunk n (PE + ACT only;
                            # emitted inside the PE-dense chunk stream)
                            for mt in range(3):
                                bp = bps.tile([128, 512], F32, tag="bcp")
                                nc.tensor.matmul(
                                    bp[:], ones_row[:],
                                    rn_sb[mt][:, 512 * n:512 * (n + 1)],
                                    start=True, stop=True)
                                bcb = bw.tile([128, 512], BF16, tag="bcs",
                                              bufs=4, name="bcs")
                                nc.scalar.copy(bcb[:], bp[:])
                                bcb_store[(mt, n)] = bcb

                        def emit_rope1(n):
                            # chunk-wise rope + in-place rstd scale so
                            # roped is complete when the pipeline drains
                            lo = 512 * n
                            for mt in (2, 0, 1):
                                is_q = mt < 2
                                x = cn[mt]
                                A_ = ropes["Aq"] if is_q else ropes["Ak"]
                                B_ = ropes["Bq"] if is_q else ropes["Bk"]
                                sh = bw.tile([128, 512], BF16, tag="shift",
                                             bufs=3, name="sh")
                                nc.gpsimd.dma_start(sh[0:64, :],
                                                    x[64:128, lo:lo + 512])
                                nc.gpsimd.dma_start(sh[64:128, :],
                                                    x[0:64, lo:lo + 512])
                                nc.vector.tensor_mul(sh[:], sh[:],
                                                     B_[:, lo:lo + 512])
                                ro = roped[(b, mt)]
                                nc.vector.tensor_mul(ro[:, lo:lo + 512],
                                                     x[:, lo:lo + 512],
                                                     A_[:, lo:lo + 512])
                                nc.vector.tensor_add(ro[:, lo:lo + 512],
                                                     ro[:, lo:lo + 512],
                                                     sh[:])
                                nc.vector.tensor_mul(
                                    ro[:, lo:lo + 512],
                                    ro[:, lo:lo + 512],
                                    bcb_store.pop((mt, n))[:])

                        sqs = {}
                        bcb_store = {}
                        for n in range(NCB + 2):
                            if n < NCB:
                                emit_chunk_mms(n)
                            if 1 <= n <= NCB:
                                emit_norm(n - 1)
                            if n >= 2:
                                emit_bc(n - 2)
                            if n < NCB:
                                emit_canon(n)
                            if n >= 2:
                                emit_rope1(n - 2)

            # ============ attention + head-split all-to-all ============
            wpool_ctx = tc.tile_pool(name="wpool", bufs=1)
            wpool = wpool_ctx.__enter__()
            # Wo resident prefetch (needed only for the output projection)
            wo_sb = wpool.tile([128, 16 * D], BF16, tag="wosb")
            wov = wo_sb[:].rearrange("p (g o) -> p g o", o=D)
            for gg in range(4):
                nc.sync.dma_start(
                    wov[:, 4 * gg:4 * (gg + 1), :],
                    woT.ap()[512 * gg:512 * (gg + 1), :]
                    .rearrange("(g p) o -> p g o", p=128))

            a2a_in = {}
            a2a_out = {}
            oin = {}
            for h in range(2):
                a2a_in[h] = dram.tile([1024, 512], BF16, tag=f"a2ain{h}",
                                      name=f"a2a_in{h}")
                a2a_out[h] = dram.tile([1024, 512], BF16, tag=f"a2aout{h}",
                                       name=f"a2a_out{h}")
                oin[h] = wpool.tile([128, 8 * 512], BF16, tag=f"oin{h}",
                                    name=f"oin{h}")

            with tc.tile_pool(name="scps", bufs=2, space="PSUM") as scps, \
                 tc.tile_pool(name="pvps", bufs=2, space="PSUM") as pvps, \
                 tc.tile_pool(name="dnps", bufs=1, space="PSUM") as dnps, \
                 tc.tile_pool(name="bcps", bufs=1, space="PSUM") as bcps, \
                 tc.tile_pool(name="apool", bufs=1) as apool:
                pending = [None]

                def flush_tail():
                    if pending[0] is None:
                        return
                    pv, dn, h, b, j = pending[0]
                    pending[0] = None
                    rec = apool.tile([1, 512], F32, tag="rec",
                                     bufs=2, name="rec")
                    nc.vector.reciprocal_approx_fast(rec[:], dn[:])
                    bc = bcps.tile([128, 512], F32, tag="bc", name="bc")
                    nc.tensor.matmul(bc[:], ones_row[:], rec[:],
                                     start=True, stop=True)
                    bcb = apool.tile([128, 512], BF16, tag="bcbn",
                                     bufs=2, name="bcb")
                    nc.vector.tensor_copy(bcb[:], bc[:])
                    nrm = apool.tile([128, 512], BF16, tag="nrm",
                                     bufs=2, name="nrm")
                    nc.vector.tensor_mul(nrm[:], pv[:], bcb[:])
                    nc.sync.dma_start(
                        a2a_in[h][128 * (4 * b + j):
                                  128 * (4 * b + j + 1), :],
                        nrm[:])

                for h in range(2):
                    for b in range(B):
                        KT = roped[(b, 2)]
                        QT = roped[(b, h)]
                        vt = vT[b]
                        for j in range(NCB):
                            pv = pvps.tile([128, 512], F32, tag="pv",
                                           name="pv")
                            dn = dnps.tile([1, 512], F32, tag="dn",
                                           name="dn")
                            acc = apool.tile([128, 512], BF16, tag="acc",
                                             bufs=2, name="acc")
                            nprs = 2 * j + 2   # pairs of Sk blocks
                            pts = [None] * nprs
                            offp = [None] * nprs

                            def emit_qk(pr):
                                ps = scps.tile([128, 1024], F32, tag="sc",
                                               name="ps")
                                pt = apool.tile([128, 1024], BF16, tag="p",
                                                bufs=4, name="pt")
                                offs = []
                                for half in range(2):
                                    i = 2 * pr + half
                                    r = i - 4 * j
                                    off = 128 * max(r, 0)
                                    offs.append(off)
                                    base = 512 * half
                                    diag = (r >= 0)
                                    nc.tensor.matmul(
                                        ps[:, base + off:base + 512],
                                        KT[:, 128 * i:128 * (i + 1)],
                                        QT[:, 512 * j + off:512 * (j + 1)],
                                        start=True, stop=not diag)
                                    if diag:
                                        nc.tensor.matmul(
                                            ps[:, base + off:
                                               base + off + 128],
                                            mask_sb[:], id_sb[:],
                                            start=False, stop=True,
                                            skip_group_check=True)
                                if offs[1] > 0:
                                    nc.tensor.matmul(
                                        ps[:, 512:512 + offs[1]],
                                        negrow[:], onesb[:, 0:offs[1]],
                                        start=True, stop=True)
                                nc.scalar.activation(
                                    pt[:, offs[0]:1024],
                                    ps[:, offs[0]:1024], AF.Exp)
                                pts[pr] = pt
                                offp[pr] = offs
                                # denominator partials on DVE
                                if pr == 0:
                                    nc.vector.tensor_copy(
                                        acc[:], pt[:, 0:512])
                                else:
                                    nc.vector.tensor_add(
                                        acc[:, offs[0]:512],
                                        acc[:, offs[0]:512],
                                        pt[:, offs[0]:512])
                                nc.vector.tensor_add(
                                    acc[:, offs[1]:512],
                                    acc[:, offs[1]:512],
                                    pt[:, 512 + offs[1]:1024])

                            def emit_pv(pr):
                                pt = pts[pr]
                                offs = offp[pr]
                                for half in range(2):
                                    i = 2 * pr + half
                                    off = offs[half]
                                    first = (i == 0)
                                    last = (i == 4 * j + 3)
                                    base = 512 * half
                                    nc.tensor.matmul(
                                        pv[:, off:512],
                                        vt[:, 128 * i:128 * (i + 1)],
                                        pt[:, base + off:base + 512],
                                        start=first, stop=last,
                                        skip_group_check=True)

                            for pr in range(nprs):
                                emit_qk(pr)
                                if pr == 0:
                                    flush_tail()
                                if pr >= 1:
                                    emit_pv(pr - 1)
                            emit_pv(nprs - 1)
                            nc.tensor.matmul(dn[:], ones_col[:], acc[:],
                                             start=True, stop=True)
                            pending[0] = (pv, dn, h, b, j)
                    flush_tail()
                    nc.gpsimd.collective_compute(
                        "AllToAll", ALU.bypass,
                        replica_groups=[list(range(N_CORES))],
                        ins=[a2a_in[h].opt()], outs=[a2a_out[h].opt()],
                        cc_dim="Partition")
                    # gpsimd queue: keeps the sync queue free for the
                    # h=1 staging DMAs while the collective runs
                    nc.gpsimd.dma_start(
                        oin[h][:].rearrange("p (s t) -> p s t", t=512),
                        a2a_out[h][:].rearrange("(s p) t -> p s t", p=128))

            # ====================== out projection ====================
            # pass A: h=0 partial sums for all (n, mp) -> SBUF (runs during
            # the second all-to-all); pass B: h=1 partials + DVE combine.
            ovs = {h: oin[h][:].rearrange("p (s t) -> p s t", t=512)
                   for h in range(2)}
            with tc.tile_pool(name="opool", bufs=1) as opool, \
                 tc.tile_pool(name="ops", bufs=2, space="PSUM") as ops:
                ph0 = {}
                for n in range(4):
                    for mp in range(4):
                        pso = ops.tile([128, 512], F32, tag=f"oa{mp}",
                                       name=f"oa{mp}")
                        for s in range(8):
                            nc.tensor.matmul(
                                pso[:],
                                ovs[0][:, s, 128 * mp:128 * (mp + 1)],
                                wov[:, 2 * s, 512 * n:512 * (n + 1)],
                                start=(s == 0), stop=(s == 7))
                        pt0 = opool.tile([128, 512], F32, tag="ph0",
                                         bufs=16, name="pt0")
                        nc.scalar.copy(pt0[:], pso[:])
                        ph0[(n, mp)] = pt0
                for n in range(4):
                    for mp in range(4):
                        pso = ops.tile([128, 512], F32, tag=f"oa{mp}",
                                       name=f"ob{mp}")
                        for s in range(8):
                            nc.tensor.matmul(
                                pso[:],
                                ovs[1][:, s, 128 * mp:128 * (mp + 1)],
                                wov[:, 2 * s + 1, 512 * n:512 * (n + 1)],
                                start=(s == 0), stop=(s == 7))
                        os_t = opool.tile([128, 512], F32, tag="osb",
                                          bufs=4, name="os_t")
                        nc.vector.tensor_add(os_t[:], pso[:],
                                             ph0[(n, mp)][:])
                        nc.sync.dma_start(
                            out.ap()[128 * mp:128 * (mp + 1),
                                     512 * n:512 * (n + 1)],
                            os_t[:])
            wpool_ctx.__exit__(None, None, None)

    nc.compile()
    return nc


_NC_CACHE = None


def _get_nc():
    global _NC_CACHE
    if _NC_CACHE is None:
        _NC_CACHE = _build()
    return _NC_CACHE


def _host_prep(inputs):
    hs = np.asarray(inputs["hidden_states"], dtype=np.float32)
    Wq = np.asarray(inputs["Wq"], dtype=np.float32)
    Wk = np.asarray(inputs["Wk"], dtype=np.float32)
    Wv = np.asarray(inputs["Wv"], dtype=np.float32)
    Wo = np.asarray(inputs["Wo"], dtype=np.float32)
    cqw = np.asarray(inputs["canon_q_w"], dtype=np.float32)
    ckw = np.asarray(inputs["canon_k_w"], dtype=np.float32)
    cvw = np.asarray(inputs["canon_v_w"], dtype=np.float32)
    qnw = np.asarray(inputs["q_norm_w"], dtype=np.float32)
    knw = np.asarray(inputs["k_norm_w"], dtype=np.float32)

    bf = ml_dtypes.bfloat16
    hsT = np.ascontiguousarray(
        np.concatenate([hs[0].T, hs[1].T], axis=1)).astype(bf)
    WqT, WkT, WvT = Wq.T, Wk.T, Wv.T
    woT = np.ascontiguousarray(Wo.T).astype(bf)

    inv_freq = 1.0 / (10000.0 ** (np.arange(0, DH, 2, dtype=np.float64) / DH))
    freqs = np.arange(S, dtype=np.float64)[:, None] * inv_freq
    emb = np.concatenate([freqs, freqs], axis=-1)
    cosT, sinT = np.cos(emb).T, np.sin(emb).T

    def make_rope(normw, scale):
        A = cosT * normw[:, None] * scale
        wswap = normw[(np.arange(DH) + 64) % DH]
        sign = np.where(np.arange(DH) < 64, -1.0, 1.0)
        Bc = sinT * wswap[:, None] * sign[:, None] * scale
        return (np.ascontiguousarray(A).astype(bf),
                np.ascontiguousarray(Bc).astype(bf))

    Aq, Bq = make_rope(qnw, SCALE)
    Ak, Bk = make_rope(knw, 1.0)

    p = np.arange(128)[:, None]
    f = np.arange(128)[None, :]
    maskd = np.where(p <= f, 0.0, NEG).astype(np.float32)
    maskTb = np.ascontiguousarray(maskd.T).astype(bf)
    idb = np.eye(128, dtype=np.float32).astype(bf)

    in_maps = []
    for r in range(N_CORES):
        wTc = np.ascontiguousarray(np.concatenate(
            [WqT[:, 256 * r:256 * r + 256],
             WkT[:, 128 * r:128 * r + 128],
             WvT[:, 128 * r:128 * r + 128]], axis=1)).astype(bf)
        cwc = np.ascontiguousarray(np.concatenate(
            [cqw[256 * r:256 * r + 256],
             ckw[128 * r:128 * r + 128],
             cvw[128 * r:128 * r + 128]], axis=0)).astype(np.float32)
        in_maps.append({
            "hsT": hsT, "wT": wTc, "woT": woT, "cw": cwc,
            "ropeAq": Aq, "ropeBq": Bq, "ropeAk": Ak, "ropeBk": Bk,
            "maskTb": maskTb, "idb": idb,
        })
    return in_maps


def kernel(**inputs):
    nc = _get_nc()
    in_maps = _host_prep(inputs)
    res = run_bass_kernel_spmd(nc, in_maps, core_ids=list(range(N_CORES)))
    full = np.empty((B, S, D), np.float32)
    for r in range(N_CORES):
        full[r // 4, 512 * (r % 4):512 * (r % 4 + 1), :] = res.results[r]["out"]
    return full


# revision 35
# speedup vs baseline: 1.6781x; 1.0467x over previous
"""Trainium2 Bass kernel for CanonCausalMultiheadAttn.

Sharding: tensor-parallel over heads across 8 cores (2 q-heads + 1 kv-head
per core), both batches replicated. Two head-split AllToAlls exchange
attention outputs so each core owns one (batch, seq-slice) of the final
output projection; the first overlaps the second half of attention and the
second overlaps the h=0 partial pass of the output projection.

Per-core pipeline (shapes hardcoded for B=2, S=2048, D=2048):
  QKV proj (bf16 matmul, weights SBUF-resident) -> canon conv via halo'd
  raw buffer (DVE, bf16) -> qk rmsnorm rstd via PE column-sum matmuls and
  fast-approx reciprocal -> RoPE (DVE bf16; norm-weight & 1/sqrt(dh)
  folded into host tables; q AND k rstd folded in via K=1 broadcast
  matmuls whose emission is software-pipelined into later PE-dense
  phases) -> causal attention with scores in [Sk, Sq] layout, two
  Sk-blocks paired per [128,1024] PSUM tile so one exp covers both; the
  causal mask and the fully-masked wedge are added on the PE itself
  (maskT.T @ I accumulation) so exp depends only on the PE; PV computed
  transposed (V stationary) directly in [dh, Sq]; softmax denominator via
  ones-column matmuls; normalize tail software-pipelined into the next
  block -> AllToAll x2 (head-split) -> output projection in two passes
  (h=0 partials to SBUF during the second AllToAll, then h=1 + combine).
"""
import sys

sys.path.insert(0, '/opt/trn_rl_repo')

import numpy as np
import ml_dtypes

import concourse.bass as bass
import concourse.mybir as mybir
import concourse.tile as tile
from concourse import bacc
from concourse.bass_utils import run_bass_kernel_spmd

F32 = mybir.dt.float32
F32R = mybir.dt.float32r
BF16 = mybir.dt.bfloat16
AF = mybir.ActivationFunctionType
ALU = mybir.AluOpType

B, S, D = 2, 2048, 2048
NH, NKV, DH = 16, 8, 128
K_CONV = 4
EPS = 1e-6
SCALE = 1.0 / float(np.sqrt(DH))
NEG = -1e9
N_CORES = 8
NCB = S // 512          # 512-token chunks per batch
N_SKB = S // 128        # Sk blocks per batch


def _build():
    nc = bacc.Bacc("TRN2", target_bir_lowering=False, debug=False,
                   num_devices=N_CORES)

    hsT = nc.dram_tensor("hsT", [D, B * S], BF16, kind="ExternalInput")
    wT = nc.dram_tensor("wT", [D, 512], BF16, kind="ExternalInput")
    woT = nc.dram_tensor("woT", [D, D], BF16, kind="ExternalInput")
    cw = nc.dram_tensor("cw", [512, K_CONV], F32, kind="ExternalInput")
    ropeAq = nc.dram_tensor("ropeAq", [DH, S], BF16, kind="ExternalInput")
    ropeBq = nc.dram_tensor("ropeBq", [DH, S], BF16, kind="ExternalInput")
    ropeAk = nc.dram_tensor("ropeAk", [DH, S], BF16, kind="ExternalInput")
    ropeBk = nc.dram_tensor("ropeBk", [DH, S], BF16, kind="ExternalInput")
    maskTb = nc.dram_tensor("maskTb", [128, 128], BF16, kind="ExternalInput")
    idb = nc.dram_tensor("idb", [128, 128], BF16, kind="ExternalInput")
    out = nc.dram_tensor("out", [512, D], F32, kind="ExternalOutput")

    with tile.TileContext(nc) as tc:
        with tc.tile_pool(name="const", bufs=1) as cpool, \
             tc.tile_pool(name="persist", bufs=1) as pers, \
             tc.tile_pool(name="dram", bufs=1, space="DRAM") as dram:

            # QKV weights resident in SBUF: [128, 16 k-blocks x 512]
            wT_sb = cpool.tile([128, 16 * 512], BF16, tag="wTsb")
            wv = wT_sb[:].rearrange("p (k c) -> p k c", c=512)
            for kk in range(4):
                nc.sync.dma_start(
                    wv[:, 4 * kk:4 * (kk + 1), :],
                    wT.ap()[512 * kk:512 * (kk + 1), :]
                    .rearrange("(k p) c -> p k c", p=128))

            # ---- constants (rope tables last: not needed until late) ----
            mask_sb = cpool.tile([128, 128], BF16, tag="mask")
            nc.sync.dma_start(mask_sb[:], maskTb.ap())
            id_sb = cpool.tile([128, 128], BF16, tag="idsb")
            nc.sync.dma_start(id_sb[:], idb.ap())
            cw_sb = []
            for mt in range(4):
                t = cpool.tile([128, K_CONV], F32, tag=f"cw{mt}",
                               name=f"cw{mt}")
                nc.sync.dma_start(t[:], cw.ap()[128 * mt:128 * mt + 128, :])
                cw_sb.append(t)
            ones_col = cpool.tile([128, 1], BF16, tag="oc")
            nc.vector.memset(ones_col[:], 1.0)
            eps_sb = cpool.tile([1, 1], F32, tag="eps")
            nc.vector.memset(eps_sb[:], EPS)
            ones_row = cpool.tile([1, 128], F32, tag="or")
            nc.vector.memset(ones_row[:], 1.0)
            # NEG broadcast: negrow.T @ onesb[:, :w] == NEG everywhere
            negrow = cpool.tile([128, 128], BF16, tag="negrow")
            nc.vector.memset(negrow[:], 0.0)
            nc.vector.memset(negrow[0:1, :], NEG)
            onesb = cpool.tile([128, 512], BF16, tag="onesb")
            nc.vector.memset(onesb[:], 1.0)
            s0_sb = []
            for mt in range(4):
                t = cpool.tile([128, 1], F32, tag=f"s0{mt}", name=f"s0{mt}")
                nc.vector.tensor_scalar_add(t[:], cw_sb[mt][:, 0:1], 1.0)
                s0_sb.append(t)
            ropes = {}
            for nm in ("Aq", "Bq", "Ak", "Bk"):
                ropes[nm] = cpool.tile([DH, S], BF16, tag=f"rope{nm}",
                                       name=f"rope{nm}")

            # persistent per-(b,mt) tiles
            roped = {}   # (b, mt) -> [128, S] bf16 (rstd folded in)
            vT = {}      # b -> [128, N_SKB*128] bf16 (V transposed blocks)

            for b in range(B):
                vT[b] = pers.tile([128, N_SKB * 128], BF16, tag=f"vT{b}",
                                  name=f"vT{b}")
                for mt in range(3):
                    roped[(b, mt)] = pers.tile(
                        [128, S], BF16, tag=f"roped{b}{mt}",
                        name=f"roped{b}{mt}")

            # ============ phase Q: QKV + canon + norm + rope ============
            # attention working tiles live at top level so they never
            # land on recycled phase-Q scratch space (avoids end-of-phase
            # write-after-read stalls)
            atop_ctx = tc.tile_pool(name="atop", bufs=1)
            atop = atop_ctx.__enter__()

            with tc.tile_pool(name="qps", bufs=1, space="PSUM") as qps, \
                 tc.tile_pool(name="spp", bufs=2, space="PSUM") as spp, \
                 tc.tile_pool(name="bps", bufs=2, space="PSUM") as bps, \
                 tc.tile_pool(name="bwork", bufs=1) as bw:
                cn = {}
                raw_h = {}
                for mt in range(4):
                    cn[mt] = bw.tile([128, S], BF16, tag=f"cn{mt}",
                                     name=f"cn{mt}")
                    raw_h[mt] = bw.tile([128, 516], BF16,
                                        tag=f"rawh{mt}",
                                        name=f"raw_h{mt}")
                psums = [qps.tile([128, 512], F32, tag=f"qk{mt}",
                                  name=f"qk{mt}") for mt in range(4)]
                for b in range(B):
                    if True:
                        for mt in range(4):
                            nc.vector.memset(raw_h[mt][:, 0:4], 0.0)

                        def emit_chunk_mms(n):
                            hs_sb = bw.tile([128, 16 * 512], BF16,
                                            tag="hschunk", bufs=2,
                                            name="hs_sb")
                            nc.sync.dma_start(
                                hs_sb[:].rearrange("p (k s) -> p k s", s=512),
                                hsT.ap()[:, b * S + 512 * n:
                                         b * S + 512 * (n + 1)]
                                .rearrange("(k p) s -> p k s", p=128))
                            hvv = hs_sb[:].rearrange("p (k s) -> p k s",
                                                     s=512)
                            for k in range(16):
                                for mt in range(4):
                                    nc.tensor.matmul(
                                        psums[mt][:],
                                        wv[:, k, 128 * mt:128 * (mt + 1)],
                                        hvv[:, k, :],
                                        start=(k == 0), stop=(k == 15))
                            if b == 0 and n == 1:
                                for nm, t in (("Aq", ropeAq), ("Bq", ropeBq),
                                              ("Ak", ropeAk), ("Bk", ropeBk)):
                                    nc.sync.dma_start(ropes[nm][:], t.ap())

                        def emit_canon(n):
                            lo = 512 * n
                            for mt in range(4):
                                rh = raw_h[mt]
                                if n > 0:
                                    nc.vector.tensor_copy(
                                        rh[:, 1:4], rh[:, 513:516])
                                nc.vector.tensor_copy(rh[:, 4:516],
                                                      psums[mt][:])
                                c = cn[mt]
                                nc.vector.tensor_scalar_mul(
                                    c[:, lo:lo + 512], rh[:, 4:516],
                                    s0_sb[mt][:])
                                for k in range(1, K_CONV):
                                    nc.vector.scalar_tensor_tensor(
                                        c[:, lo:lo + 512],
                                        rh[:, 4 - k:516 - k],
                                        cw_sb[mt][:, k:k + 1],
                                        c[:, lo:lo + 512],
                                        ALU.mult, ALU.add)
                            # squares for rmsnorm (q0, q1, k)
                            for mt in range(3):
                                sq = bw.tile([128, 512], BF16, tag="sqr",
                                             bufs=3, name="sq")
                                nc.vector.tensor_mul(
                                    sq[:], cn[mt][:, lo:lo + 512],
                                    cn[mt][:, lo:lo + 512])
                                sqs[(n, mt)] = sq

                        def emit_norm(n):
                            for mt in range(3):
                                sp = spp.tile([1, 512], F32, tag="ssq")
                                nc.tensor.matmul(sp[:], ones_col[:],
                                                 sqs.pop((n, mt))[:],
                                                 start=True, stop=True)
                                srt = bw.tile([1, 512], F32, tag="srt",
                                              bufs=2, name="srt")
                                nc.scalar.activation(srt[:], sp[:], AF.Sqrt,
                                                     bias=eps_sb[:],
                                                     scale=1.0 / DH)
                                rn = bw.tile([1, 512], F32, tag=f"rn{mt}",
                                             bufs=3, name=f"rn{mt}")
                                nc.vector.reciprocal_approx_fast(
                                    rn[:], srt[:])
                                rns[(n, mt)] = rn
                            # V transpose blocks for this chunk
                            for t in range(4):
                                i = 4 * n + t
                                nc.sync.dma_start_transpose(
                                    vT[b][:, 128 * i:128 * (i + 1)],
                                    cn[3][:, 128 * i:128 * (i + 1)])

                        def emit_bc(n):
                            # rstd broadcast for chunk n (PE + ACT only;
                            # emitted inside the PE-dense chunk stream)
                            for mt in range(3):
                                bp = bps.tile([128, 512], F32, tag="bcp")
                                nc.tensor.matmul(
                                    bp[:], ones_row[:],
                                    rns.pop((n, mt))[:],
                                    start=True, stop=True)
                                bcb = bw.tile([128, 512], BF16, tag="bcs",
                                              bufs=4, name="bcs")
                                nc.scalar.copy(bcb[:], bp[:])
                                bcb_store[(mt, n)] = bcb

                        def emit_rope1(n):
                            # chunk-wise rope + in-place rstd scale so
                            # roped is complete when the pipeline drains
                            lo = 512 * n
                            for mt in (2, 0, 1):
                                is_q = mt < 2
                                x = cn[mt]
                                A_ = ropes["Aq"] if is_q else ropes["Ak"]
                                B_ = ropes["Bq"] if is_q else ropes["Bk"]
                                sh = bw.tile([128, 512], BF16, tag="shift",
                                             bufs=3, name="sh")
                                nc.gpsimd.dma_start(sh[0:64, :],
                                                    x[64:128, lo:lo + 512])
                                nc.gpsimd.dma_start(sh[64:128, :],
                                                    x[0:64, lo:lo + 512])
                                nc.vector.tensor_mul(sh[:], sh[:],
                                                     B_[:, lo:lo + 512])
                                ro = roped[(b, mt)]
                                nc.vector.tensor_mul(ro[:, lo:lo + 512],
                                                     x[:, lo:lo + 512],
                                                     A_[:, lo:lo + 512])
                                nc.vector.tensor_add(ro[:, lo:lo + 512],
                                                     ro[:, lo:lo + 512],
                                                     sh[:])
                                nc.vector.tensor_mul(
                                    ro[:, lo:lo + 512],
                                    ro[:, lo:lo + 512],
                                    bcb_store.pop((mt, n))[:])

                        sqs = {}
                        rns = {}
                        bcb_store = {}
                        for n in range(NCB + 2):
                            if n < NCB:
                                emit_chunk_mms(n)
                            if 1 <= n <= NCB:
                                emit_norm(n - 1)
                            if n >= 2:
                                emit_bc(n - 2)
                            if n < NCB:
                                emit_canon(n)
                            if n >= 2:
                                emit_rope1(n - 2)

            # ============ attention + head-split all-to-all ============
            wpool_ctx = tc.tile_pool(name="wpool", bufs=1)
            wpool = wpool_ctx.__enter__()
            # Wo resident prefetch (needed only for the output projection)
            wo_sb = wpool.tile([128, 16 * D], BF16, tag="wosb")
            wov = wo_sb[:].rearrange("p (g o) -> p g o", o=D)
            for gg in range(4):
                nc.sync.dma_start(
                    wov[:, 4 * gg:4 * (gg + 1), :],
                    woT.ap()[512 * gg:512 * (gg + 1), :]
                    .rearrange("(g p) o -> p g o", p=128))

            a2a_in = {}
            a2a_out = {}
            oin = {}
            for h in range(2):
                a2a_in[h] = dram.tile([1024, 512], BF16, tag=f"a2ain{h}",
                                      name=f"a2a_in{h}")
                a2a_out[h] = dram.tile([1024, 512], BF16, tag=f"a2aout{h}",
                                       name=f"a2a_out{h}")
                oin[h] = wpool.tile([128, 8 * 512], BF16, tag=f"oin{h}",
                                    name=f"oin{h}")

            with tc.tile_pool(name="scps", bufs=2, space="PSUM") as scps, \
                 tc.tile_pool(name="pvps", bufs=2, space="PSUM") as pvps, \
                 tc.tile_pool(name="dnps", bufs=1, space="PSUM") as dnps, \
                 tc.tile_pool(name="bcps", bufs=1, space="PSUM") as bcps:
                pending = [None]

                def flush_tail():
                    if pending[0] is None:
                        return
                    pv, dn, h, b, j = pending[0]
                    pending[0] = None
                    rec = atop.tile([1, 512], F32, tag="rec",
                                     bufs=2, name="rec")
                    nc.vector.reciprocal_approx_fast(rec[:], dn[:])
                    bc = bcps.tile([128, 512], F32, tag="bc", name="bc")
                    nc.tensor.matmul(bc[:], ones_row[:], rec[:],
                                     start=True, stop=True)
                    bcb = atop.tile([128, 512], BF16, tag="bcbn",
                                     bufs=2, name="bcb")
                    nc.vector.tensor_copy(bcb[:], bc[:])
                    nrm = atop.tile([128, 512], BF16, tag="nrm",
                                     bufs=2, name="nrm")
                    nc.vector.tensor_mul(nrm[:], pv[:], bcb[:])
                    nc.sync.dma_start(
                        a2a_in[h][128 * (4 * b + j):
                                  128 * (4 * b + j + 1), :],
                        nrm[:])

                for h in range(2):
                    for b in range(B):
                        KT = roped[(b, 2)]
                        QT = roped[(b, h)]
                        vt = vT[b]
                        for j in range(NCB):
                            pv = pvps.tile([128, 512], F32, tag="pv",
                                           name="pv")
                            dn = dnps.tile([1, 512], F32, tag="dn",
                                           name="dn")
                            acc = atop.tile([128, 512], BF16, tag="acc",
                                             bufs=2, name="acc")
                            nprs = 2 * j + 2   # pairs of Sk blocks
                            pts = [None] * nprs
                            offp = [None] * nprs

                            def emit_qk(pr):
                                ps = scps.tile([128, 1024], F32, tag="sc",
                                               name="ps")
                                pt = atop.tile([128, 1024], BF16, tag="p",
                                                bufs=4, name="pt")
                                offs = []
                                for half in range(2):
                                    i = 2 * pr + half
                                    r = i - 4 * j
                                    off = 128 * max(r, 0)
                                    offs.append(off)
                                    base = 512 * half
                                    diag = (r >= 0)
                                    nc.tensor.matmul(
                                        ps[:, base + off:base + 512],
                                        KT[:, 128 * i:128 * (i + 1)],
                                        QT[:, 512 * j + off:512 * (j + 1)],
                                        start=True, stop=not diag)
                                    if diag:
                                        nc.tensor.matmul(
                                            ps[:, base + off:
                                               base + off + 128],
                                            mask_sb[:], id_sb[:],
                                            start=False, stop=True,
                                            skip_group_check=True)
                                if offs[1] > 0:
                                    nc.tensor.matmul(
                                        ps[:, 512:512 + offs[1]],
                                        negrow[:], onesb[:, 0:offs[1]],
                                        start=True, stop=True)
                                nc.scalar.activation(
                                    pt[:, offs[0]:1024],
                                    ps[:, offs[0]:1024], AF.Exp)
                                pts[pr] = pt
                                offp[pr] = offs
                                # denominator partials on DVE
                                if pr == 0:
                                    nc.vector.tensor_copy(
                                        acc[:], pt[:, 0:512])
                                else:
                                    nc.vector.tensor_add(
                                        acc[:, offs[0]:512],
                                        acc[:, offs[0]:512],
                                        pt[:, offs[0]:512])
                                nc.vector.tensor_add(
                                    acc[:, offs[1]:512],
                                    acc[:, offs[1]:512],
                                    pt[:, 512 + offs[1]:1024])

                            def emit_pv(pr):
                                pt = pts[pr]
                                offs = offp[pr]
                                for half in range(2):
                                    i = 2 * pr + half
                                    off = offs[half]
                                    first = (i == 0)
                                    last = (i == 4 * j + 3)
                                    base = 512 * half
                                    nc.tensor.matmul(
                                        pv[:, off:512],
                                        vt[:, 128 * i:128 * (i + 1)],
                                        pt[:, base + off:base + 512],
                                        start=first, stop=last,
                                        skip_group_check=True)

                            for pr in range(nprs):
                                emit_qk(pr)
                                if pr == 0:
                                    flush_tail()
                                if pr >= 1:
                                    emit_pv(pr - 1)
                            emit_pv(nprs - 1)
                            nc.tensor.matmul(dn[:], ones_col[:], acc[:],
                                             start=True, stop=True)
                            pending[0] = (pv, dn, h, b, j)
                    flush_tail()
                    nc.gpsimd.collective_compute(
                        "AllToAll", ALU.bypass,
                        replica_groups=[list(range(N_CORES))],
                        ins=[a2a_in[h].opt()], outs=[a2a_out[h].opt()],
                        cc_dim="Partition")
                    # gpsimd queue: keeps the sync queue free for the
                    # h=1 staging DMAs while the collective runs
                    nc.gpsimd.dma_start(
                        oin[h][:].rearrange("p (s t) -> p s t", t=512),
                        a2a_out[h][:].rearrange("(s p) t -> p s t", p=128))

            # ====================== out projection ====================
            # pass A: h=0 partial sums for all (n, mp) -> SBUF (runs during
            # the second all-to-all); pass B: h=1 partials + DVE combine.
            ovs = {h: oin[h][:].rearrange("p (s t) -> p s t", t=512)
                   for h in range(2)}
            with tc.tile_pool(name="opool", bufs=1) as opool, \
                 tc.tile_pool(name="ops", bufs=2, space="PSUM") as ops:
                ph0 = {}
                for n in range(4):
                    for mp in range(4):
                        pso = ops.tile([128, 512], F32, tag=f"oa{mp}",
                                       name=f"oa{mp}")
                        for s in range(8):
                            nc.tensor.matmul(
                                pso[:],
                                ovs[0][:, s, 128 * mp:128 * (mp + 1)],
                                wov[:, 2 * s, 512 * n:512 * (n + 1)],
                                start=(s == 0), stop=(s == 7))
                        pt0 = opool.tile([128, 512], F32, tag="ph0",
                                         bufs=16, name="pt0")
                        nc.scalar.copy(pt0[:], pso[:])
                        ph0[(n, mp)] = pt0
                for n in range(4):
                    for mp in range(4):
                        pso = ops.tile([128, 512], F32, tag=f"oa{mp}",
                                       name=f"ob{mp}")
                        for s in range(8):
                            nc.tensor.matmul(
                                pso[:],
                                ovs[1][:, s, 128 * mp:128 * (mp + 1)],
                                wov[:, 2 * s + 1, 512 * n:512 * (n + 1)],
                                start=(s == 0), stop=(s == 7))
                        os_t = opool.tile([128, 512], F32, tag="osb",
                                          bufs=4, name="os_t")
                        nc.vector.tensor_add(os_t[:], pso[:],
                                             ph0[(n, mp)][:])
                        nc.sync.dma_start(
                            out.ap()[128 * mp:128 * (mp + 1),
                                     512 * n:512 * (n + 1)],
                            os_t[:])
            wpool_ctx.__exit__(None, None, None)
            atop_ctx.__exit__(None, None, None)

    nc.compile()
    return nc


_NC_CACHE = None


def _get_nc():
    global _NC_CACHE
    if _NC_CACHE is None:
        _NC_CACHE = _build()
    return _NC_CACHE


def _host_prep(inputs):
    hs = np.asarray(inputs["hidden_states"], dtype=np.float32)
    Wq = np.asarray(inputs["Wq"], dtype=np.float32)
    Wk = np.asarray(inputs["Wk"], dtype=np.float32)
    Wv = np.asarray(inputs["Wv"], dtype=np.float32)
    Wo = np.asarray(inputs["Wo"], dtype=np.float32)
    cqw = np.asarray(inputs["canon_q_w"], dtype=np.float32)
    ckw = np.asarray(inputs["canon_k_w"], dtype=np.float32)
    cvw = np.asarray(inputs["canon_v_w"], dtype=np.float32)
    qnw = np.asarray(inputs["q_norm_w"], dtype=np.float32)
    knw = np.asarray(inputs["k_norm_w"], dtype=np.float32)

    bf = ml_dtypes.bfloat16
    hsT = np.ascontiguousarray(
        np.concatenate([hs[0].T, hs[1].T], axis=1)).astype(bf)
    WqT, WkT, WvT = Wq.T, Wk.T, Wv.T
    woT = np.ascontiguousarray(Wo.T).astype(bf)

    inv_freq = 1.0 / (10000.0 ** (np.arange(0, DH, 2, dtype=np.float64) / DH))
    freqs = np.arange(S, dtype=np.float64)[:, None] * inv_freq
    emb = np.concatenate([freqs, freqs], axis=-1)
    cosT, sinT = np.cos(emb).T, np.sin(emb).T

    def make_rope(normw, scale):
        A = cosT * normw[:, None] * scale
        wswap = normw[(np.arange(DH) + 64) % DH]
        sign = np.where(np.arange(DH) < 64, -1.0, 1.0)
        Bc = sinT * wswap[:, None] * sign[:, None] * scale
        return (np.ascontiguousarray(A).astype(bf),
                np.ascontiguousarray(Bc).astype(bf))

    Aq, Bq = make_rope(qnw, SCALE)
    Ak, Bk = make_rope(knw, 1.0)

    p = np.arange(128)[:, None]
    f = np.arange(128)[None, :]
    maskd = np.where(p <= f, 0.0, NEG).astype(np.float32)
    maskTb = np.ascontiguousarray(maskd.T).astype(bf)
    idb = np.eye(128, dtype=np.float32).astype(bf)

    in_maps = []
    for r in range(N_CORES):
        wTc = np.ascontiguousarray(np.concatenate(
            [WqT[:, 256 * r:256 * r + 256],
             WkT[:, 128 * r:128 * r + 128],
             WvT[:, 128 * r:128 * r + 128]], axis=1)).astype(bf)
        cwc = np.ascontiguousarray(np.concatenate(
            [cqw[256 * r:256 * r + 256],
             ckw[128 * r:128 * r + 128],
             cvw[128 * r:128 * r + 128]], axis=0)).astype(np.float32)
        in_maps.append({
            "hsT": hsT, "wT": wTc, "woT": woT, "cw": cwc,
            "ropeAq": Aq, "ropeBq": Bq, "ropeAk": Ak, "ropeBk": Bk,
            "maskTb": maskTb, "idb": idb,
        })
    return in_maps


def kernel(**inputs):
    nc = _get_nc()
    in_maps = _host_prep(inputs)
    res = run_bass_kernel_spmd(nc, in_maps, core_ids=list(range(N_CORES)))
    full = np.empty((B, S, D), np.float32)
    for r in range(N_CORES):
        full[r // 4, 512 * (r % 4):512 * (r % 4 + 1), :] = res.results[r]["out"]
    return full
